# revision 11
# baseline (speedup 1.0000x reference)
"""Trainium2 Bass kernel for the ConfusionClassification criterion.

Computes, for full inputs
    pred_logits      [64, 65536, 2] f32
    pred_confusion   [64, 65536, 4] f32
    target_classes   [64, 65536]    int64 (values 0/1)
the scalar loss
    src  = argmax(pred_logits, -1)
    c    = g==1 ? (src==1 ? 1 : 2) : (src==1 ? 3 : 0)
    loss = mean_{b,n}( logsumexp(pred_confusion) - pred_confusion[c] )

Sharding: pure data-parallel over the batch dim; each of the 8 NeuronCores
processes 8 batches (524288 points).  Each core emits per-partition partial
sums of logsumexp and of the selected logit; the host reduces them.

Per-core device pipeline (per tile of 128 x L points):
  DMA   : conf [128,4L], logits [128,2L], target-as-f32 [128,L]
  ACT   : e = exp(conf)                      (4L elems)
  DVE   : s = (e0+e1)+(e2+e3)                (3 tensor_tensor adds)
  DVE   : p = is_gt(l1, l0)                  (argmax as mask)
  DVE   : 3x copy_predicated, in-place on the conf tile:
            X0 <- X3 where p ; X2 <- X1 where p ; X0 <- X2 where g
          leaving X0 = pred_confusion[c]
  ACT   : Ln(s)   with accum_out -> sum(lse)  per partition
  ACT   : Copy(X0) with accum_out -> sum(sel) per partition
"""

import sys
import types

for _p in ("/opt/trn_rl_repo",):
    if _p not in sys.path:
        sys.path.insert(0, _p)

import numpy as np


def _install_ntff_hook_shim():
    """This image's antenv lacks axon_hooks, so trn_boot's NTFF profile hook
    registration degrades silently and bass_utils crashes on import if tracing
    is requested (e.g. BASS_TRACE=1).  Recreate the module and register the
    ctypes hook trn_boot would have installed.  No-op if the module exists."""
    try:
        import antenv.axon_hooks  # noqa: F401

        return
    except ImportError:
        pass
    try:
        import antenv
        from trn_agent_boot.trn_boot import _ntff_profile_via_ctypes
    except ImportError:
        return
    mod = types.ModuleType("antenv.axon_hooks")
    mod._hook = None
    mod.set_axon_ntff_profile_hook = lambda h: setattr(mod, "_hook", h)
    mod.get_axon_ntff_profile_hook = lambda: mod._hook
    sys.modules["antenv.axon_hooks"] = mod
    antenv.axon_hooks = mod
    try:
        mod._hook = _ntff_profile_via_ctypes("/opt/axon/libaxon_pjrt.so")
    except Exception:
        pass


_install_ntff_hook_shim()

import concourse.bacc as bacc
import concourse.bass as bass
import concourse.mybir as mybir
from concourse.bass_utils import run_bass_kernel_spmd
from concourse.mybir import AluOpType
from concourse.tile import TileContext

AF = mybir.ActivationFunctionType
F32 = mybir.dt.float32
U8 = mybir.dt.uint8
I8 = mybir.dt.int8

P = 128
B, N = 64, 65536
M = 8                      # cores
BS = B // M                # batches per core
NP_CORE = BS * N           # points per core
T = 4                      # tiles per core
L = NP_CORE // (T * P)     # points per partition per tile


def emit_loss_kernel(
    nc, conf, lg, tgt, out_acc, n_tiles, width, io_bufs=2, tmp_bufs=2, repeat=1,
    ops=frozenset({"exp", "sums", "pm", "sel", "ln", "selsum"}),
):
    """Emit the per-core loss kernel.

    conf: DRAM AP [T, 128, 4L]  interleaved 4-class confusion logits
    lg:   DRAM AP [T, 128, 2L]  interleaved 2-class prediction logits
    tgt:  DRAM AP [T, 128, L]   target class as uint8 0/1
    out_acc: DRAM AP [128, 2T]  cols [0,T) = sum(lse), cols [T,2T) = sum(sel)
    repeat: re-run the whole pass this many times (benchmark differencing)
    """
    with TileContext(nc) as tc:
        with (
            tc.tile_pool(name="io", bufs=io_bufs) as io_pool,
            tc.tile_pool(name="tmp", bufs=tmp_bufs) as tmp_pool,
            tc.tile_pool(name="accp", bufs=1) as acc_pool,
        ):
            acc = acc_pool.tile([P, 2 * n_tiles], F32)
            if not ({"ln", "selsum"} & ops):
                nc.vector.memset(acc[:], 0.0)
            for _rep, t in ((r, t) for r in range(repeat) for t in range(n_tiles)):
                conf_t = io_pool.tile([P, 4 * width], F32, tag="conf")
                lg_t = io_pool.tile([P, 2 * width], F32, tag="lg")
                tgt_t = io_pool.tile([P, width], U8, tag="tgt")
                nc.sync.dma_start(out=conf_t[:], in_=conf[t])
                nc.sync.dma_start(out=lg_t[:], in_=lg[t])
                nc.sync.dma_start(out=tgt_t[:], in_=tgt[t])

                e_t = tmp_pool.tile([P, 4 * width], F32, tag="e")
                t01 = tmp_pool.tile([P, width], F32, tag="t01")
                t23 = tmp_pool.tile([P, width], F32, tag="t23")
                s = tmp_pool.tile([P, width], F32, tag="s")
                pm = tmp_pool.tile([P, width], I8, tag="pm")

                X = conf_t[:].rearrange("p (l k) -> p l k", k=4)
                E = e_t[:].rearrange("p (l k) -> p l k", k=4)
                LG = lg_t[:].rearrange("p (l k) -> p l k", k=2)

                if "exp" in ops:
                    nc.scalar.activation(e_t[:], conf_t[:], AF.Exp)
                if "sums" in ops:
                    nc.vector.tensor_tensor(t01[:], E[:, :, 0], E[:, :, 1], AluOpType.add)
                    nc.vector.tensor_tensor(t23[:], E[:, :, 2], E[:, :, 3], AluOpType.add)
                    nc.vector.tensor_tensor(s[:], t01[:], t23[:], AluOpType.add)
                if "pm" in ops:
                    nc.vector.tensor_tensor(pm[:], LG[:, :, 1], LG[:, :, 0], AluOpType.is_gt)
                if "sel" in ops:
                    nc.vector.copy_predicated(X[:, :, 0], pm[:], X[:, :, 3])
                    nc.vector.copy_predicated(X[:, :, 2], pm[:], X[:, :, 1])
                    nc.vector.copy_predicated(X[:, :, 0], tgt_t[:], X[:, :, 2])
                if "ln" in ops:
                    nc.scalar.activation(
                        t01[:], s[:], AF.Ln, accum_out=acc[:, t : t + 1]
                    )
                if "selsum" in ops:
                    nc.scalar.activation(
                        t23[:], X[:, :, 0], AF.Copy,
                        accum_out=acc[:, n_tiles + t : n_tiles + t + 1],
                    )
            nc.sync.dma_start(out=out_acc, in_=acc[:])
    return nc


def _pin_act_table_set(nc, set_id):
    """Replace the alternating per-function ACT table loads with a single
    load of one set that contains every function the kernel uses.

    bacc's insert_act_table_loads greedily picks the first act_info.json set
    containing each function, so an Exp/Ln/Copy mix thrashes between
    exp_and_others and natural_log -- ~2.7us per reload, serialized on ACT.
    natural_log_exp_and_others (set 6 on gen3) holds Exp, Ln and Copy, so one
    load suffices.  The inserted loads carry no sync_info, so dropping the
    extras cannot break semaphore bookkeeping.
    """
    for fn in nc.m.functions:
        for blk in fn.blocks:
            first = True
            keep = []
            for ins in blk.instructions:
                if isinstance(ins, mybir.InstLoadActFuncSet):
                    assert ins.sync_info is None or (
                        not ins.sync_info.on_wait and not ins.sync_info.on_update
                    )
                    if not first:
                        continue
                    ins.act_func_set_id = set_id
                    first = False
                keep.append(ins)
            if len(keep) != len(blk.instructions):
                blk.instructions[:] = keep


def build_nc(n_tiles=T, width=L, io_bufs=2, tmp_bufs=2, repeat=1,
             ops=frozenset({"exp", "sums", "pm", "sel", "ln", "selsum"})):
    nc = bacc.Bacc("TRN2", target_bir_lowering=False, debug=False)
    conf = nc.dram_tensor("conf", [n_tiles, P, 4 * width], F32, kind="ExternalInput").ap()
    lg = nc.dram_tensor("lg", [n_tiles, P, 2 * width], F32, kind="ExternalInput").ap()
    tgt = nc.dram_tensor("tgt", [n_tiles, P, width], U8, kind="ExternalInput").ap()
    out_acc = nc.dram_tensor("acc", [P, 2 * n_tiles], F32, kind="ExternalOutput").ap()
    emit_loss_kernel(
        nc, conf, lg, tgt, out_acc, n_tiles, width, io_bufs, tmp_bufs, repeat, ops
    )
    nc.finalize()
    _pin_act_table_set(nc, 6)
    return nc


BF16 = mybir.dt.bfloat16
I16 = mybir.dt.int16


def emit_loss_kernel_v2(
    nc, conf, lg, tgt, out_acc, n_tiles, width, io_bufs=3, tmp_bufs=2, repeat=1,
    ops=frozenset({"exp", "sums", "pm", "sel", "ln", "selsum"}),
):
    """Planar bf16 variant.

    conf: DRAM AP [T, 128, 4L] bf16, free dim = [class k][point l] (planar)
    lg:   DRAM AP [T, 128, 2L] bf16 planar
    tgt:  DRAM AP [T, 128, L]  uint8
    out_acc: DRAM AP [128, T+1]: cols [0,T) per-partition sum(lse); entry
      [0, T] = total sum(sel) (from the PE column-sum path, partition 0 only).

    All DVE ops are step-1 16-bit -> 2x_1P mode.  sum(sel) is computed by the
    TensorEngine as a ones-vector column sum accumulated in PSUM across tiles,
    then one ACT Copy+accum over [1, L] at the end.
    """
    W = width
    with TileContext(nc) as tc:
        with (
            tc.tile_pool(name="io", bufs=io_bufs) as io_pool,
            tc.tile_pool(name="tmp", bufs=tmp_bufs) as tmp_pool,
            tc.tile_pool(name="accp", bufs=1) as acc_pool,
            tc.tile_pool(name="psum", bufs=1, space="PSUM") as psum_pool,
        ):
            acc = acc_pool.tile([P, 2 * n_tiles + 1], F32)
            ones = acc_pool.tile([P, 1], BF16)
            nc.vector.memset(ones[:], 1.0)
            nc.vector.memset(acc[:], 0.0)
            selp = psum_pool.tile([1, W], F32)
            n_chunks = (W + 511) // 512
            total = repeat * n_tiles
            it = 0
            for _rep, t in ((r, t) for r in range(repeat) for t in range(n_tiles)):
                conf_t = io_pool.tile([P, 4 * W], BF16, tag="conf")
                lg_t = io_pool.tile([P, 2 * W], BF16, tag="lg")
                tgt_t = io_pool.tile([P, W], U8, tag="tgt")
                nc.sync.dma_start(out=conf_t[:], in_=conf[t])
                nc.sync.dma_start(out=lg_t[:], in_=lg[t])
                nc.sync.dma_start(out=tgt_t[:], in_=tgt[t])

                e_t = tmp_pool.tile([P, 4 * W], BF16, tag="e")
                s01 = tmp_pool.tile([P, W], BF16, tag="s01")
                s23 = tmp_pool.tile([P, W], BF16, tag="s23")
                s = tmp_pool.tile([P, W], BF16, tag="s")
                pm = tmp_pool.tile([P, W], I16, tag="pm")

                ca = conf_t[:]
                ea = e_t[:]
                la = lg_t[:]
                Xs = [ca[:, k * W : (k + 1) * W] for k in range(4)]
                Es = [ea[:, k * W : (k + 1) * W] for k in range(4)]
                L0 = la[:, 0:W]
                L1 = la[:, W : 2 * W]

                if "pm" in ops:
                    nc.vector.tensor_tensor(pm[:], L1, L0, AluOpType.is_gt)
                if "exp" in ops:
                    nc.scalar.activation(e_t[:], conf_t[:], AF.Exp)
                if "sums" in ops:
                    eng01 = nc.gpsimd if "pooladds" in ops else nc.vector
                    eng01.tensor_tensor(s01[:], Es[0], Es[1], AluOpType.add)
                    eng01.tensor_tensor(s23[:], Es[2], Es[3], AluOpType.add)
                    nc.vector.tensor_tensor(s[:], s01[:], s23[:], AluOpType.add)
                if "sel" in ops:
                    nc.vector.copy_predicated(Xs[0], pm[:], Xs[3])
                    nc.vector.copy_predicated(Xs[2], pm[:], Xs[1])
                    nc.vector.copy_predicated(Xs[0], tgt_t[:], Xs[2])
                if "ln" in ops:
                    nc.scalar.activation(
                        s01[:], s[:], AF.Ln, accum_out=acc[:, t : t + 1]
                    )
                if "selsum" in ops:
                    if "selsum_pe" in ops:
                        for c in range(n_chunks):
                            lo, hi = c * 512, min((c + 1) * 512, W)
                            nc.tensor.matmul(
                                selp[:, lo:hi],
                                ones[:],
                                Xs[0][:, lo:hi],
                                start=(it == 0),
                                stop=(it == total - 1),
                            )
                    else:
                        nc.scalar.activation(
                            s23[:], Xs[0], AF.Copy,
                            accum_out=acc[:, n_tiles + 1 + t : n_tiles + 2 + t],
                        )
                it += 1
            if "selsum" in ops and "selsum_pe" in ops:
                seljunk = acc_pool.tile([1, W], F32)
                nc.scalar.activation(
                    seljunk[:],
                    selp[:, :],
                    AF.Copy,
                    accum_out=acc[0:1, n_tiles : n_tiles + 1],
                )
            nc.sync.dma_start(out=out_acc, in_=acc[:])
    return nc


def build_nc_v2(n_tiles=T, width=None, io_bufs=3, tmp_bufs=2, repeat=1,
                ops=frozenset({"exp", "sums", "pm", "sel", "ln", "selsum"})):
    if width is None:
        width = NP_CORE // (n_tiles * P)
    nc = bacc.Bacc("TRN2", target_bir_lowering=False, debug=False)
    conf = nc.dram_tensor("conf", [n_tiles, P, 4 * width], BF16, kind="ExternalInput").ap()
    lg = nc.dram_tensor("lg", [n_tiles, P, 2 * width], BF16, kind="ExternalInput").ap()
    tgt = nc.dram_tensor("tgt", [n_tiles, P, width], U8, kind="ExternalInput").ap()
    out_acc = nc.dram_tensor("acc", [P, 2 * n_tiles + 1], F32, kind="ExternalOutput").ap()
    emit_loss_kernel_v2(
        nc, conf, lg, tgt, out_acc, n_tiles, width, io_bufs, tmp_bufs, repeat, ops
    )
    nc.finalize()
    _pin_act_table_set(nc, 6)
    return nc


def shard_inputs_v2(pred_logits, pred_confusion, target_classes, n_tiles=T, width=None):
    import ml_dtypes

    if width is None:
        width = NP_CORE // (n_tiles * P)
    T_, L_ = n_tiles, width
    bf16 = ml_dtypes.bfloat16
    in_maps = []
    for i in range(M):
        sl = slice(i * BS, (i + 1) * BS)
        conf = (
            np.asarray(pred_confusion[sl], dtype=np.float32)
            .reshape(T_, P, L_, 4)
            .transpose(0, 1, 3, 2)
            .astype(bf16)
            .reshape(T_, P, 4 * L_)
        )
        conf = np.ascontiguousarray(conf)
        lg = (
            np.asarray(pred_logits[sl], dtype=np.float32)
            .reshape(T_, P, L_, 2)
            .transpose(0, 1, 3, 2)
            .astype(bf16)
            .reshape(T_, P, 2 * L_)
        )
        lg = np.ascontiguousarray(lg)
        tgt = np.asarray(target_classes[sl], dtype=np.uint8).reshape(T_, P, L_)
        in_maps.append({"conf": conf, "lg": lg, "tgt": tgt})
    return in_maps


def shard_inputs(pred_logits, pred_confusion, target_classes):
    in_maps = []
    for i in range(M):
        sl = slice(i * BS, (i + 1) * BS)
        conf = np.ascontiguousarray(pred_confusion[sl], dtype=np.float32).reshape(
            T, P, 4 * L
        )
        lg = np.ascontiguousarray(pred_logits[sl], dtype=np.float32).reshape(
            T, P, 2 * L
        )
        tgt = np.asarray(target_classes[sl], dtype=np.uint8).reshape(T, P, L)
        in_maps.append({"conf": conf, "lg": lg, "tgt": tgt})
    return in_maps


FP8 = mybir.dt.float8e4


def emit_loss_kernel_v3(
    nc, conf, lg, tgt, out_acc, n_tiles, width, io_bufs=3, tmp_bufs=2, repeat=1,
    pool_s=True,
):
    """fp8 planar variant with minimal DVE work.

    conf: DRAM AP [T, 128, 4W] fp8, planes ordered [x2 | x0 | x1 | x3]
    lg:   DRAM AP [T, 128, 2W] bf16, planes [l0 | l1]
    tgt:  DRAM AP [T, 128, W]  uint8 (0/1)
    out_acc: DRAM AP [128, T+1]: cols [0,T) per-partition sum(lse(tile));
      entry [0, T] = total sum(selected logit) via the PE column-sum path.

    Engine split per tile:
      ACT : E = exp(conf) [4W], Ln(s)+accum [W]
      DVE : s2 = E[:2W]+E[2W:], pm = l1>l0, 3 copy_predicated (selection)
      Pool: s = s2[:W]+s2[W:]
      PE  : psum += ones^T @ sel  (column sums, accumulated across tiles)
    """
    W = width
    with TileContext(nc) as tc:
        with (
            tc.tile_pool(name="io", bufs=io_bufs) as io_pool,
            tc.tile_pool(name="tmp", bufs=tmp_bufs) as tmp_pool,
            tc.tile_pool(name="accp", bufs=1) as acc_pool,
            tc.tile_pool(name="psum", bufs=1, space="PSUM") as psum_pool,
        ):
            acc = acc_pool.tile([P, n_tiles + 1], F32)
            ones = acc_pool.tile([P, 1], BF16)
            nc.vector.memset(ones[:], 1.0)
            nc.vector.memset(acc[:], 0.0)
            selp = psum_pool.tile([1, W], F32)
            n_chunks = (W + 511) // 512
            total = repeat * n_tiles
            it = 0
            for _rep, t in ((r, t) for r in range(repeat) for t in range(n_tiles)):
                conf_t = io_pool.tile([P, 4 * W], FP8, tag="conf")
                lg_t = io_pool.tile([P, 2 * W], BF16, tag="lg")
                tgt_t = io_pool.tile([P, W], U8, tag="tgt")
                nc.sync.dma_start(out=conf_t[:], in_=conf[t])
                nc.sync.dma_start(out=lg_t[:], in_=lg[t])
                nc.sync.dma_start(out=tgt_t[:], in_=tgt[t])

                e_t = tmp_pool.tile([P, 4 * W], BF16, tag="e")
                s2 = tmp_pool.tile([P, 2 * W], BF16, tag="s2")
                s = tmp_pool.tile([P, W], BF16, tag="s")
                pm = tmp_pool.tile([P, W], I16, tag="pm")

                ca = conf_t[:]
                A = ca[:, : 2 * W]          # [x2 | x0]
                B = ca[:, 2 * W :]          # [x1 | x3]
                ea = e_t[:]

                nc.scalar.activation(e_t[:], conf_t[:], AF.Exp)
                nc.vector.tensor_tensor(
                    s2[:], ea[:, : 2 * W], ea[:, 2 * W :], AluOpType.add
                )
                eng_s = nc.gpsimd if pool_s else nc.vector
                eng_s.tensor_tensor(s[:], s2[:, :W], s2[:, W:], AluOpType.add)
                nc.vector.tensor_tensor(pm[:], lg_t[:, W:], lg_t[:, :W], AluOpType.is_gt)
                # selection: A[:, :W] = pm ? x1 : x2 ; A[:, W:] = pm ? x3 : x0
                nc.vector.copy_predicated(A[:, :W], pm[:], B[:, :W])
                nc.vector.copy_predicated(A[:, W:], pm[:], B[:, W:])
                # g-select: sel = g ? A[:, :W] : A[:, W:]  ->  A[:, W:]
                nc.vector.copy_predicated(A[:, W:], tgt_t[:], A[:, :W])
                nc.scalar.activation(
                    s2[:, :W], s[:], AF.Ln, accum_out=acc[:, t : t + 1]
                )
                for c in range(n_chunks):
                    lo, hi = c * 512, min((c + 1) * 512, W)
                    nc.tensor.matmul(
                        selp[:, lo:hi],
                        ones[:],
                        A[:, W + lo : W + hi],
                        start=(it == 0),
                        stop=(it == total - 1),
                    )
                it += 1
            seljunk = acc_pool.tile([1, W], F32)
            nc.scalar.activation(
                seljunk[:],
                selp[:, :],
                AF.Copy,
                accum_out=acc[0:1, n_tiles : n_tiles + 1],
            )
            nc.sync.dma_start(out=out_acc, in_=acc[:])
    return nc


def build_nc_v3(n_tiles=T, width=None, io_bufs=3, tmp_bufs=2, repeat=1, pool_s=True):
    if width is None:
        width = NP_CORE // (n_tiles * P)
    nc = bacc.Bacc("TRN2", target_bir_lowering=False, debug=False)
    conf = nc.dram_tensor("conf", [n_tiles, P, 4 * width], FP8, kind="ExternalInput").ap()
    lg = nc.dram_tensor("lg", [n_tiles, P, 2 * width], BF16, kind="ExternalInput").ap()
    tgt = nc.dram_tensor("tgt", [n_tiles, P, width], U8, kind="ExternalInput").ap()
    out_acc = nc.dram_tensor("acc", [P, n_tiles + 1], F32, kind="ExternalOutput").ap()
    emit_loss_kernel_v3(
        nc, conf, lg, tgt, out_acc, n_tiles, width, io_bufs, tmp_bufs, repeat, pool_s
    )
    nc.finalize()
    _pin_act_table_set(nc, 6)
    return nc


def shard_inputs_v3(pred_logits, pred_confusion, target_classes, n_tiles=T, width=None):
    import ml_dtypes

    if width is None:
        width = NP_CORE // (n_tiles * P)
    T_, L_ = n_tiles, width
    bf16 = ml_dtypes.bfloat16
    fp8 = ml_dtypes.float8_e4m3
    in_maps = []
    for i in range(M):
        sl = slice(i * BS, (i + 1) * BS)
        conf = (
            np.asarray(pred_confusion[sl], dtype=np.float32)
            .reshape(T_, P, L_, 4)
            .transpose(0, 1, 3, 2)[:, :, [2, 0, 1, 3], :]
            .astype(fp8)
            .reshape(T_, P, 4 * L_)
        )
        conf = np.ascontiguousarray(conf)
        lg = (
            np.asarray(pred_logits[sl], dtype=np.float32)
            .reshape(T_, P, L_, 2)
            .transpose(0, 1, 3, 2)
            .astype(bf16)
            .reshape(T_, P, 2 * L_)
        )
        lg = np.ascontiguousarray(lg)
        tgt = np.asarray(target_classes[sl], dtype=np.uint8).reshape(T_, P, L_)
        in_maps.append({"conf": conf, "lg": lg, "tgt": tgt})
    return in_maps


def reduce_v3(results, n_tiles=T):
    total = 0.0
    for r in results:
        a = np.asarray(r["acc"], dtype=np.float64)
        total += a[:, :n_tiles].sum() - a[0, n_tiles]
    return np.float32(total / (B * N))


def kernel_v3(pred_logits, pred_confusion, target_classes):
    if "nc3" not in _CACHED:
        _CACHED["nc3"] = build_nc_v3()
    in_maps = shard_inputs_v3(pred_logits, pred_confusion, target_classes)
    results = run_bass_kernel_spmd(_CACHED["nc3"], in_maps, list(range(M))).results
    return reduce_v3(results)


def emit_loss_kernel_v4(
    nc, conf, lg, tgt, out_acc, n_tiles, width, io_bufs=3, tmp_bufs=2,
    repeat=1,
):
    """Pair-interleaved fp8 variant.

    conf: DRAM AP [T, 128, 4W] fp8. Free layout per tile:
      bytes [0, 2W):  A-half, interleaved pairs (x2[i], x0[i])
      bytes [2W, 4W): B-half, interleaved pairs (x1[i], x3[i])
    lg:  DRAM AP [T, 128, 2W] bf16 planar [l0 | l1]
    tgt: DRAM AP [T, 128, W] uint8
    out_acc: [128, T+1] f32: cols [0,T) per-partition per-tile sum(ln(s));
    entry [0, T] = total sum of the selected logit

    Per tile: ACT exp (de-interleaving output AP) + Ln/accum; DVE does one
    2W add, one W add, is_gt, an int16 copy_predicated (moves both
    pm-selected fp8 bytes per point under one mask element) and a strided
    fp8 copy_predicated for the g-select; PE accumulates column sums of the
    selected values in PSUM.
    """
    W = width
    I16_ = I16
    with TileContext(nc) as tc:
        with (
            tc.tile_pool(name="io", bufs=io_bufs) as io_pool,
            tc.tile_pool(name="tmp", bufs=tmp_bufs) as tmp_pool,
            tc.tile_pool(name="accp", bufs=1) as acc_pool,
            tc.tile_pool(name="psum", bufs=1, space="PSUM") as psum_pool,
        ):
            acc = acc_pool.tile([P, n_tiles + 1], F32)
            ones = acc_pool.tile([P, 1], BF16)
            nc.vector.memset(ones[:], 1.0)
            nc.vector.memset(acc[:], 0.0)
            selp = psum_pool.tile([1, W], F32)
            n_chunks = (W + 511) // 512
            total = repeat * n_tiles
            it = 0
            for _rep, t in ((r, t) for r in range(repeat) for t in range(n_tiles)):
                conf_t = io_pool.tile([P, 4 * W], FP8, tag="conf")
                lg_t = io_pool.tile([P, 2 * W], BF16, tag="lg")
                tgt_t = io_pool.tile([P, W], U8, tag="tgt")
                nc.sync.dma_start(out=conf_t[:], in_=conf[t])
                nc.sync.dma_start(out=lg_t[:], in_=lg[t])
                nc.sync.dma_start(out=tgt_t[:], in_=tgt[t])

                e_t = tmp_pool.tile([P, 4 * W], BF16, tag="e")
                s2 = tmp_pool.tile([P, 2 * W], BF16, tag="s2")
                s = tmp_pool.tile([P, W], BF16, tag="s")
                pm = tmp_pool.tile([P, W], I16_, tag="pm")

                ca = conf_t[:]
                ea = e_t[:]
                # exp, de-interleaving via strided READS (free on ACT; strided
                # writes are catastrophically slow): input addr 2W*h + 2w + k
                # -> output addr 2W*h + W*k + w, iterated (h, k, w).
                cin = ca.rearrange("p (h w k) -> p h k w", h=2, k=2)
                eout = ea.rearrange("p (h k w) -> p h k w", h=2, k=2)
                nc.scalar.activation(eout, cin, AF.Exp)
                # e_t now planar: [e2 | e0 | e1 | e3]
                nc.vector.tensor_tensor(
                    s2[:], ea[:, : 2 * W], ea[:, 2 * W :], AluOpType.add
                )  # [e2+e1 | e0+e3]
                nc.vector.tensor_tensor(s[:], s2[:, :W], s2[:, W:], AluOpType.add)
                nc.vector.tensor_tensor(
                    pm[:], lg_t[:, W:], lg_t[:, :W], AluOpType.is_gt
                )
                # pm-select both branches at once on the int16 view:
                # A-pairs (x2,x0) <- B-pairs (x1,x3) where pm
                c16 = ca.bitcast(I16_)
                nc.vector.copy_predicated(c16[:, :W], pm[:], c16[:, W:])
                # g-select: odd bytes (x0 slot) <- even bytes (x2 slot) where g
                apairs = ca[:, : 2 * W].rearrange("p (w k) -> p k w", k=2)
                sel_ap = apairs[:, 1]
                nc.vector.copy_predicated(sel_ap, tgt_t[:], apairs[:, 0])
                nc.scalar.activation(
                    s2[:, :W], s[:], AF.Ln, accum_out=acc[:, t : t + 1]
                )
                for c in range(n_chunks):
                    lo, hi = c * 512, min((c + 1) * 512, W)
                    nc.tensor.matmul(
                        selp[:, lo:hi],
                        ones[:],
                        sel_ap[:, lo:hi],
                        start=(it == 0),
                        stop=(it == total - 1),
                    )
                it += 1
            seljunk = acc_pool.tile([1, W], F32)
            nc.scalar.activation(
                seljunk[:], selp[:, :], AF.Copy,
                accum_out=acc[0:1, n_tiles : n_tiles + 1],
            )
            nc.sync.dma_start(out=out_acc, in_=acc[:])
    return nc


def build_nc_v4(n_tiles=T, width=None, io_bufs=3, tmp_bufs=2, repeat=1):
    if width is None:
        width = NP_CORE // (n_tiles * P)
    nc = bacc.Bacc("TRN2", target_bir_lowering=False, debug=False)
    conf = nc.dram_tensor("conf", [n_tiles, P, 4 * width], FP8, kind="ExternalInput").ap()
    lg = nc.dram_tensor("lg", [n_tiles, P, 2 * width], BF16, kind="ExternalInput").ap()
    tgt = nc.dram_tensor("tgt", [n_tiles, P, width], U8, kind="ExternalInput").ap()
    out_acc = nc.dram_tensor("acc", [P, n_tiles + 1], F32, kind="ExternalOutput").ap()
    emit_loss_kernel_v4(
        nc, conf, lg, tgt, out_acc, n_tiles, width, io_bufs, tmp_bufs, repeat
    )
    nc.finalize()
    _pin_act_table_set(nc, 6)
    return nc


def shard_inputs_v4(pred_logits, pred_confusion, target_classes, n_tiles=T, width=None):
    import ml_dtypes

    if width is None:
        width = NP_CORE // (n_tiles * P)
    T_, L_ = n_tiles, width
    bf16 = ml_dtypes.bfloat16
    fp8 = ml_dtypes.float8_e4m3
    in_maps = []
    for i in range(M):
        sl = slice(i * BS, (i + 1) * BS)
        c = np.asarray(pred_confusion[sl], dtype=np.float32).reshape(T_, P, L_, 4)
        conf = np.empty((T_, P, 4 * L_), dtype=fp8)
        ch = conf.reshape(T_, P, 2, L_, 2)
        ch[:, :, 0, :, 0] = c[..., 2]
        ch[:, :, 0, :, 1] = c[..., 0]
        ch[:, :, 1, :, 0] = c[..., 1]
        ch[:, :, 1, :, 1] = c[..., 3]
        lg = (
            np.asarray(pred_logits[sl], dtype=np.float32)
            .reshape(T_, P, L_, 2)
            .transpose(0, 1, 3, 2)
            .astype(bf16)
            .reshape(T_, P, 2 * L_)
        )
        lg = np.ascontiguousarray(lg)
        tgt = np.asarray(target_classes[sl], dtype=np.uint8).reshape(T_, P, L_)
        in_maps.append({"conf": conf, "lg": lg, "tgt": tgt})
    return in_maps


def reduce_v4(results, n_tiles=T):
    total = 0.0
    for r in results:
        a = np.asarray(r["acc"], dtype=np.float64)
        total += a[:, :n_tiles].sum() - a[0, n_tiles]
    return np.float32(total / (B * N))


def kernel_v4(pred_logits, pred_confusion, target_classes):
    if "nc4" not in _CACHED:
        _CACHED["nc4"] = build_nc_v4()
    in_maps = shard_inputs_v4(pred_logits, pred_confusion, target_classes)
    results = run_bass_kernel_spmd(_CACHED["nc4"], in_maps, list(range(M))).results
    return reduce_v4(results)


V5_WIDTHS = (512, 1024, 1024, 1024, 512)


def emit_loss_kernel_v5(nc, conf, lg, tgt, out_acc, widths, io_bufs=4, tmp_bufs=2):
    """v4 pipeline with nonuniform tile widths (small first tile so ACT can
    start sooner; small last tile so the tail chain is short).

    conf: DRAM AP [P, 4*sum(W)] fp8; tile t occupies columns
      [4*off_t, 4*off_t + 4*W_t), laid out [A-half pairs | B-half pairs]
    lg:   DRAM AP [P, 2*sum(W)] bf16; tile block [l0 plane | l1 plane]
    tgt:  DRAM AP [P, sum(W)] u8
    out_acc: [P, T+1] f32 (cols 0..T-1: per-tile ln sums; [0,T]: sel sum)
    """
    T_ = len(widths)
    offs = [0]
    for w in widths:
        offs.append(offs[-1] + w)
    Wmax = max(widths)
    with TileContext(nc) as tc:
        with (
            tc.tile_pool(name="io", bufs=io_bufs) as io_pool,
            tc.tile_pool(name="tmp", bufs=tmp_bufs) as tmp_pool,
            tc.tile_pool(name="accp", bufs=1) as acc_pool,
            tc.tile_pool(name="psum", bufs=1, space="PSUM") as psum_pool,
        ):
            acc = acc_pool.tile([P, T_ + 1], F32)
            ones = acc_pool.tile([P, 1], BF16)
            nc.vector.memset(ones[:], 1.0)
            nc.vector.memset(acc[:], 0.0)
            selp = psum_pool.tile([1, 512], F32)
            n_chunk_total = sum((w + 511) // 512 for w in widths)
            ic = 0
            for t, W in enumerate(widths):
                off = offs[t]
                conf_t = io_pool.tile([P, 4 * Wmax], FP8, tag="conf")
                lg_t = io_pool.tile([P, 2 * Wmax], BF16, tag="lg")
                tgt_t = io_pool.tile([P, Wmax], U8, tag="tgt")
                nc.sync.dma_start(
                    out=conf_t[:, : 4 * W], in_=conf[:, 4 * off : 4 * (off + W)]
                )
                nc.sync.dma_start(
                    out=lg_t[:, : 2 * W], in_=lg[:, 2 * off : 2 * (off + W)]
                )
                nc.sync.dma_start(out=tgt_t[:, :W], in_=tgt[:, off : off + W])

                e_t = tmp_pool.tile([P, 4 * Wmax], BF16, tag="e")
                s2 = tmp_pool.tile([P, 2 * Wmax], BF16, tag="s2")
                s = tmp_pool.tile([P, Wmax], BF16, tag="s")
                pm = tmp_pool.tile([P, Wmax], I16, tag="pm")

                ca = conf_t[:, : 4 * W]
                ea = e_t[:, : 4 * W]
                cin = ca.rearrange("p (h w k) -> p h k w", h=2, k=2)
                eout = ea.rearrange("p (h k w) -> p h k w", h=2, k=2)
                nc.scalar.activation(eout, cin, AF.Exp)
                nc.vector.tensor_tensor(
                    s2[:, : 2 * W], ea[:, : 2 * W], ea[:, 2 * W :], AluOpType.add
                )
                nc.vector.tensor_tensor(
                    s[:, :W], s2[:, :W], s2[:, W : 2 * W], AluOpType.add
                )
                nc.vector.tensor_tensor(
                    pm[:, :W], lg_t[:, W : 2 * W], lg_t[:, :W], AluOpType.is_gt
                )
                c16 = conf_t[:].bitcast(I16)
                nc.vector.copy_predicated(c16[:, :W], pm[:, :W], c16[:, W : 2 * W])
                apairs = ca[:, : 2 * W].rearrange("p (w k) -> p k w", k=2)
                sel_ap = apairs[:, 1]
                nc.vector.copy_predicated(sel_ap, tgt_t[:, :W], apairs[:, 0])
                nc.scalar.activation(
                    s2[:, :W], s[:, :W], AF.Ln, accum_out=acc[:, t : t + 1]
                )
                for c in range((W + 511) // 512):
                    lo, hi = c * 512, min((c + 1) * 512, W)
                    nc.tensor.matmul(
                        selp[:, : hi - lo],
                        ones[:],
                        sel_ap[:, lo:hi],
                        start=(ic == 0),
                        stop=(ic == n_chunk_total - 1),
                    )
                    ic += 1
            seljunk = acc_pool.tile([1, 512], F32)
            nc.scalar.activation(
                seljunk[:], selp[:, :], AF.Copy,
                accum_out=acc[0:1, T_ : T_ + 1],
            )
            nc.sync.dma_start(out=out_acc, in_=acc[:])
    return nc


def build_nc_v5(widths=V5_WIDTHS, io_bufs=4, tmp_bufs=2):
    total = sum(widths)
    assert total * P == NP_CORE
    nc = bacc.Bacc("TRN2", target_bir_lowering=False, debug=False)
    conf = nc.dram_tensor("conf", [P, 4 * total], FP8, kind="ExternalInput").ap()
    lg = nc.dram_tensor("lg", [P, 2 * total], BF16, kind="ExternalInput").ap()
    tgt = nc.dram_tensor("tgt", [P, total], U8, kind="ExternalInput").ap()
    out_acc = nc.dram_tensor(
        "acc", [P, len(widths) + 1], F32, kind="ExternalOutput"
    ).ap()
    emit_loss_kernel_v5(nc, conf, lg, tgt, out_acc, widths, io_bufs, tmp_bufs)
    nc.finalize()
    _pin_act_table_set(nc, 6)
    return nc


def shard_inputs_v5(pred_logits, pred_confusion, target_classes, widths=V5_WIDTHS):
    import ml_dtypes

    bf16 = ml_dtypes.bfloat16
    fp8 = ml_dtypes.float8_e4m3
    total = sum(widths)
    offs = [0]
    for w in widths:
        offs.append(offs[-1] + w)
    in_maps = []
    for i in range(M):
        sl = slice(i * BS, (i + 1) * BS)
        c = np.asarray(pred_confusion[sl], dtype=np.float32).reshape(P, total, 4)
        lgf = np.asarray(pred_logits[sl], dtype=np.float32).reshape(P, total, 2)
        tgf = np.asarray(target_classes[sl], dtype=np.uint8).reshape(P, total)
        conf = np.empty((P, 4 * total), dtype=fp8)
        lg = np.empty((P, 2 * total), dtype=bf16)
        for t, W in enumerate(widths):
            off = offs[t]
            cb = c[:, off : off + W, :]
            blk = conf[:, 4 * off : 4 * (off + W)].reshape(P, 2, W, 2)
            blk[:, 0, :, 0] = cb[..., 2]
            blk[:, 0, :, 1] = cb[..., 0]
            blk[:, 1, :, 0] = cb[..., 1]
            blk[:, 1, :, 1] = cb[..., 3]
            lb = lg[:, 2 * off : 2 * (off + W)].reshape(P, 2, W)
            lb[:, 0, :] = lgf[:, off : off + W, 0]
            lb[:, 1, :] = lgf[:, off : off + W, 1]
        in_maps.append({"conf": conf, "lg": lg, "tgt": tgf})
    return in_maps


def reduce_v5(results, widths=V5_WIDTHS):
    T_ = len(widths)
    total = 0.0
    for r in results:
        a = np.asarray(r["acc"], dtype=np.float64)
        total += a[:, :T_].sum() - a[0, T_]
    return np.float32(total / (B * N))


def kernel_v5(pred_logits, pred_confusion, target_classes):
    if "nc5" not in _CACHED:
        _CACHED["nc5"] = build_nc_v5()
    in_maps = shard_inputs_v5(pred_logits, pred_confusion, target_classes)
    results = run_bass_kernel_spmd(_CACHED["nc5"], in_maps, list(range(M))).results
    return reduce_v5(results)


def emit_loss_kernel_v6(
    nc, conf, lg, tgt, out_acc, out_sel, n_tiles, width, io_bufs=4, tmp_bufs=3,
    prod_depth=1, split_t0=True,
):
    """v4 + schedule trims.

    - tile 0's conf DMA and exp are split into A/B halves so ACT starts as
      soon as half the first tile has landed
    - ln operates on pairwise products of s (prod_depth levels), shifting
      work from ACT (1x) to DVE (2x); Sum ln(s_i) == Sum ln(prod pairs)
    - the PSUM column-sum of the selected logits is exported with a DVE
      tensor_copy + DMA instead of an ACT copy (shorter tail)
    out_acc: [P, T] f32; out_sel: [1, 512] f32 (host sums + subtracts)
    """
    W = width
    with TileContext(nc) as tc:
        with (
            tc.tile_pool(name="io", bufs=io_bufs) as io_pool,
            tc.tile_pool(name="tmp", bufs=tmp_bufs) as tmp_pool,
            tc.tile_pool(name="accp", bufs=1) as acc_pool,
            tc.tile_pool(name="psum", bufs=1, space="PSUM") as psum_pool,
        ):
            acc = acc_pool.tile([P, n_tiles], F32)
            ones = acc_pool.tile([P, 1], BF16)
            nc.vector.memset(ones[:], 1.0)
            selp = psum_pool.tile([1, W], F32)
            n_chunks = (W + 511) // 512
            for t in range(n_tiles):
                conf_t = io_pool.tile([P, 4 * W], FP8, tag="conf")
                lg_t = io_pool.tile([P, 2 * W], BF16, tag="lg")
                tgt_t = io_pool.tile([P, W], U8, tag="tgt")
                if t == 0 and split_t0:
                    nc.sync.dma_start(out=conf_t[:, : 2 * W], in_=conf[t][:, : 2 * W])
                    nc.sync.dma_start(out=conf_t[:, 2 * W :], in_=conf[t][:, 2 * W :])
                else:
                    nc.sync.dma_start(out=conf_t[:], in_=conf[t])
                nc.sync.dma_start(out=lg_t[:], in_=lg[t])
                nc.sync.dma_start(out=tgt_t[:], in_=tgt[t])

                e_t = tmp_pool.tile([P, 4 * W], BF16, tag="e")
                s2 = tmp_pool.tile([P, 2 * W], BF16, tag="s2")
                s = tmp_pool.tile([P, W], BF16, tag="s")
                pm = tmp_pool.tile([P, W], I16, tag="pm")

                ca = conf_t[:]
                ea = e_t[:]
                if t == 0 and split_t0:
                    for h in (0, 1):
                        cin = ca[:, 2 * W * h : 2 * W * (h + 1)].rearrange(
                            "p (w k) -> p k w", k=2
                        )
                        eout = ea[:, 2 * W * h : 2 * W * (h + 1)].rearrange(
                            "p (k w) -> p k w", k=2
                        )
                        nc.scalar.activation(eout, cin, AF.Exp)
                else:
                    cin = ca.rearrange("p (h w k) -> p h k w", h=2, k=2)
                    eout = ea.rearrange("p (h k w) -> p h k w", h=2, k=2)
                    nc.scalar.activation(eout, cin, AF.Exp)
                nc.vector.tensor_tensor(
                    s2[:], ea[:, : 2 * W], ea[:, 2 * W :], AluOpType.add
                )
                nc.vector.tensor_tensor(s[:], s2[:, :W], s2[:, W:], AluOpType.add)
                nc.vector.tensor_tensor(
                    pm[:], lg_t[:, W:], lg_t[:, :W], AluOpType.is_gt
                )
                c16 = ca.bitcast(I16)
                nc.vector.copy_predicated(c16[:, :W], pm[:], c16[:, W:])
                apairs = ca[:, : 2 * W].rearrange("p (w k) -> p k w", k=2)
                sel_ap = apairs[:, 1]
                nc.vector.copy_predicated(sel_ap, tgt_t[:], apairs[:, 0])
                # pairwise products: Sum ln(s) = Sum ln(prod of pairs)
                sp = tmp_pool.tile([P, W], BF16, tag="sp")
                lw = W
                bufs = (s, sp)
                for _d in range(prod_depth):
                    lw //= 2
                    src, dst = bufs[_d % 2][:], bufs[(_d + 1) % 2][:]
                    nc.vector.tensor_tensor(
                        dst[:, :lw], src[:, :lw], src[:, lw : 2 * lw],
                        AluOpType.mult,
                    )
                nc.scalar.activation(
                    s2[:, :lw], bufs[prod_depth % 2][:, :lw], AF.Ln,
                    accum_out=acc[:, t : t + 1],
                )
                for c in range(n_chunks):
                    lo, hi = c * 512, min((c + 1) * 512, W)
                    nc.tensor.matmul(
                        selp[:, lo:hi],
                        ones[:],
                        sel_ap[:, lo:hi],
                        start=(t == 0),
                        stop=(t == n_tiles - 1),
                    )
            selsb = acc_pool.tile([1, W], F32)
            nc.vector.tensor_copy(selsb[:], selp[:])
            nc.sync.dma_start(out=out_acc, in_=acc[:])
            nc.sync.dma_start(out=out_sel, in_=selsb[:])
    return nc


def build_nc_v6(n_tiles=T, width=None, io_bufs=4, tmp_bufs=3, prod_depth=1,
                split_t0=True):
    if width is None:
        width = NP_CORE // (n_tiles * P)
    nc = bacc.Bacc("TRN2", target_bir_lowering=False, debug=False)
    conf = nc.dram_tensor("conf", [n_tiles, P, 4 * width], FP8, kind="ExternalInput").ap()
    lg = nc.dram_tensor("lg", [n_tiles, P, 2 * width], BF16, kind="ExternalInput").ap()
    tgt = nc.dram_tensor("tgt", [n_tiles, P, width], U8, kind="ExternalInput").ap()
    out_acc = nc.dram_tensor("acc", [P, n_tiles], F32, kind="ExternalOutput").ap()
    out_sel = nc.dram_tensor("selv", [1, width], F32, kind="ExternalOutput").ap()
    emit_loss_kernel_v6(
        nc, conf, lg, tgt, out_acc, out_sel, n_tiles, width, io_bufs, tmp_bufs,
        prod_depth, split_t0,
    )
    nc.finalize()
    _pin_act_table_set(nc, 6)
    return nc


def reduce_v6(results, n_tiles=T):
    total = 0.0
    for r in results:
        total += np.asarray(r["acc"], dtype=np.float64).sum()
        total -= np.asarray(r["selv"], dtype=np.float64).sum()
    return np.float32(total / (B * N))


def kernel_v6(pred_logits, pred_confusion, target_classes):
    if "nc6" not in _CACHED:
        _CACHED["nc6"] = build_nc_v6()
    in_maps = shard_inputs_v4(pred_logits, pred_confusion, target_classes)
    results = run_bass_kernel_spmd(_CACHED["nc6"], in_maps, list(range(M))).results
    return reduce_v6(results)


def emit_loss_kernel_v7(
    nc, conf, lg, tgt, out_acc, n_tiles, width, tmp_bufs=2, prod_depth=1,
    split_t0=True,
):
    """All-resident variant: the whole per-core input (36 KB/partition) is
    DMA'd up front into single SBUF tensors, in tile-priority order, so the
    DMA engines run flat out from the start and compute never recycles io
    buffers.  Per tile the DVE queue is ordered [sums, products] ->
    [selection] so the ACT Ln chain unblocks as early as possible.

    conf: DRAM [P, 4*T*W] fp8 (v5 flat layout: per-tile blocks [A|B] pairs)
    lg:   DRAM [P, 2*T*W] bf16 (per-tile blocks [l0|l1])
    tgt:  DRAM [P, T*W] u8
    out_acc: [P, T+1] f32 (cols 0..T-1 ln sums; [0,T] sel sum)
    """
    W = width
    T_ = n_tiles
    with TileContext(nc) as tc:
        with (
            tc.tile_pool(name="io", bufs=1) as io_pool,
            tc.tile_pool(name="tmp", bufs=tmp_bufs) as tmp_pool,
            tc.tile_pool(name="accp", bufs=1) as acc_pool,
            tc.tile_pool(name="psum", bufs=1, space="PSUM") as psum_pool,
        ):
            acc = acc_pool.tile([P, T_ + 1], F32)
            ones = acc_pool.tile([P, 1], BF16)
            nc.vector.memset(ones[:], 1.0)
            nc.vector.memset(acc[:], 0.0)
            selp = psum_pool.tile([1, 512], F32)
            conf_all = io_pool.tile([P, 4 * T_ * W], FP8, tag="conf")
            lg_all = io_pool.tile([P, 2 * T_ * W], BF16, tag="lg")
            tgt_all = io_pool.tile([P, T_ * W], U8, tag="tgt")
            # All input DMAs up front, in the order compute consumes them.
            for t in range(T_):
                c_sb = conf_all[:, 4 * W * t : 4 * W * (t + 1)]
                c_dr = conf[:, 4 * W * t : 4 * W * (t + 1)]
                if t == 0 and split_t0:
                    nc.sync.dma_start(out=c_sb[:, : 2 * W], in_=c_dr[:, : 2 * W])
                    nc.sync.dma_start(out=c_sb[:, 2 * W :], in_=c_dr[:, 2 * W :])
                else:
                    nc.sync.dma_start(out=c_sb, in_=c_dr)
                nc.sync.dma_start(
                    out=lg_all[:, 2 * W * t : 2 * W * (t + 1)],
                    in_=lg[:, 2 * W * t : 2 * W * (t + 1)],
                )
                nc.sync.dma_start(
                    out=tgt_all[:, W * t : W * (t + 1)],
                    in_=tgt[:, W * t : W * (t + 1)],
                )
            n_chunks = (W + 511) // 512
            ic = 0
            for t in range(T_):
                ca = conf_all[:, 4 * W * t : 4 * W * (t + 1)]
                lga = lg_all[:, 2 * W * t : 2 * W * (t + 1)]
                tga = tgt_all[:, W * t : W * (t + 1)]
                e_t = tmp_pool.tile([P, 4 * W], BF16, tag="e")
                s2 = tmp_pool.tile([P, 2 * W], BF16, tag="s2")
                s = tmp_pool.tile([P, W], BF16, tag="s")
                sp = tmp_pool.tile([P, W], BF16, tag="sp")
                pm = tmp_pool.tile([P, W], I16, tag="pm")
                ea = e_t[:]
                if t == 0 and split_t0:
                    for h in (0, 1):
                        cin = ca[:, 2 * W * h : 2 * W * (h + 1)].rearrange(
                            "p (w k) -> p k w", k=2
                        )
                        eout = ea[:, 2 * W * h : 2 * W * (h + 1)].rearrange(
                            "p (k w) -> p k w", k=2
                        )
                        nc.scalar.activation(eout, cin, AF.Exp)
                else:
                    cin = ca.rearrange("p (h w k) -> p h k w", h=2, k=2)
                    eout = ea.rearrange("p (h k w) -> p h k w", h=2, k=2)
                    nc.scalar.activation(eout, cin, AF.Exp)
                # lse path first so the ACT Ln unblocks early
                nc.vector.tensor_tensor(
                    s2[:], ea[:, : 2 * W], ea[:, 2 * W :], AluOpType.add
                )
                nc.vector.tensor_tensor(s[:], s2[:, :W], s2[:, W:], AluOpType.add)
                lw = W
                bufs = (s, sp)
                for _d in range(prod_depth):
                    lw //= 2
                    src, dst = bufs[_d % 2][:], bufs[(_d + 1) % 2][:]
                    nc.vector.tensor_tensor(
                        dst[:, :lw], src[:, :lw], src[:, lw : 2 * lw],
                        AluOpType.mult,
                    )
                nc.scalar.activation(
                    s2[:, :lw], bufs[prod_depth % 2][:, :lw], AF.Ln,
                    accum_out=acc[:, t : t + 1],
                )
                # selection
                nc.vector.tensor_tensor(
                    pm[:], lga[:, W:], lga[:, :W], AluOpType.is_gt
                )
                c16 = ca.bitcast(I16)
                nc.vector.copy_predicated(c16[:, :W], pm[:], c16[:, W:])
                apairs = ca[:, : 2 * W].rearrange("p (w k) -> p k w", k=2)
                sel_ap = apairs[:, 1]
                nc.vector.copy_predicated(sel_ap, tga, apairs[:, 0])
                for c in range(n_chunks):
                    lo, hi = c * 512, min((c + 1) * 512, W)
                    nc.tensor.matmul(
                        selp[:, : hi - lo],
                        ones[:],
                        sel_ap[:, lo:hi],
                        start=(ic == 0),
                        stop=(ic == n_chunks * T_ - 1),
                    )
                    ic += 1
            seljunk = acc_pool.tile([1, 512], F32)
            nc.scalar.activation(
                seljunk[:], selp[:], AF.Copy,
                accum_out=acc[0:1, T_ : T_ + 1],
            )
            nc.sync.dma_start(out=out_acc, in_=acc[:])
    return nc


def build_nc_v7(n_tiles=T, width=None, tmp_bufs=2, prod_depth=1, split_t0=True,
                io_bufs=None):
    if width is None:
        width = NP_CORE // (n_tiles * P)
    total = n_tiles * width
    nc = bacc.Bacc("TRN2", target_bir_lowering=False, debug=False)
    conf = nc.dram_tensor("conf", [P, 4 * total], FP8, kind="ExternalInput").ap()
    lg = nc.dram_tensor("lg", [P, 2 * total], BF16, kind="ExternalInput").ap()
    tgt = nc.dram_tensor("tgt", [P, total], U8, kind="ExternalInput").ap()
    out_acc = nc.dram_tensor(
        "acc", [P, n_tiles + 1], F32, kind="ExternalOutput"
    ).ap()
    emit_loss_kernel_v7(
        nc, conf, lg, tgt, out_acc, n_tiles, width, tmp_bufs, prod_depth, split_t0
    )
    nc.finalize()
    _pin_act_table_set(nc, 6)
    return nc


def shard_inputs_v7(pred_logits, pred_confusion, target_classes, n_tiles=T):
    width = NP_CORE // (n_tiles * P)
    widths = tuple([width] * n_tiles)
    return shard_inputs_v5(
        pred_logits, pred_confusion, target_classes, widths=widths
    )


def reduce_v7(results, n_tiles=T):
    total = 0.0
    for r in results:
        a = np.asarray(r["acc"], dtype=np.float64)
        total += a[:, :n_tiles].sum() - a[0, n_tiles]
    return np.float32(total / (B * N))


def kernel_v7(pred_logits, pred_confusion, target_classes):
    if "nc7" not in _CACHED:
        _CACHED["nc7"] = build_nc_v7()
    in_maps = shard_inputs_v7(pred_logits, pred_confusion, target_classes)
    results = run_bass_kernel_spmd(_CACHED["nc7"], in_maps, list(range(M))).results
    return reduce_v7(results)


_CACHED = {}


def _get_nc():
    if "nc" not in _CACHED:
        _CACHED["nc"] = build_nc()
    return _CACHED["nc"]


def kernel(pred_logits, pred_confusion, target_classes):
    nc = _get_nc()
    in_maps = shard_inputs(pred_logits, pred_confusion, target_classes)
    results = run_bass_kernel_spmd(nc, in_maps, list(range(M))).results
    total = 0.0
    for r in results:
        a = np.asarray(r["acc"], dtype=np.float64)
        total += a[:, :T].sum() - a[:, T:].sum()
    return np.float32(total / (B * N))


def reduce_v2(results):
    total = 0.0
    for r in results:
        a = np.asarray(r["acc"], dtype=np.float64)
        total += a[:, :T].sum() - a[0, T] - a[:, T + 1 :].sum()
    return np.float32(total / (B * N))


def kernel_v2(pred_logits, pred_confusion, target_classes):
    if "nc2" not in _CACHED:
        _CACHED["nc2"] = build_nc_v2()
    in_maps = shard_inputs_v2(pred_logits, pred_confusion, target_classes)
    results = run_bass_kernel_spmd(_CACHED["nc2"], in_maps, list(range(M))).results
    return reduce_v2(results)



# revision 15
# speedup vs baseline: 1.0134x; 1.0134x over previous
"""Trainium2 Bass kernel for the ConfusionClassification criterion.

Computes, for full inputs
    pred_logits      [64, 65536, 2] f32
    pred_confusion   [64, 65536, 4] f32
    target_classes   [64, 65536]    int64 (values 0/1)
the scalar loss
    src  = argmax(pred_logits, -1)
    c    = g==1 ? (src==1 ? 1 : 2) : (src==1 ? 3 : 0)
    loss = mean_{b,n}( logsumexp(pred_confusion) - pred_confusion[c] )

Sharding: pure data-parallel over the batch dim; each of the 8 NeuronCores
processes 8 batches (524288 points).  Each core emits per-partition partial
sums of logsumexp and of the selected logit; the host reduces them.

Per-core device pipeline (per tile of 128 x L points):
  DMA   : conf [128,4L], logits [128,2L], target-as-f32 [128,L]
  ACT   : e = exp(conf)                      (4L elems)
  DVE   : s = (e0+e1)+(e2+e3)                (3 tensor_tensor adds)
  DVE   : p = is_gt(l1, l0)                  (argmax as mask)
  DVE   : 3x copy_predicated, in-place on the conf tile:
            X0 <- X3 where p ; X2 <- X1 where p ; X0 <- X2 where g
          leaving X0 = pred_confusion[c]
  ACT   : Ln(s)   with accum_out -> sum(lse)  per partition
  ACT   : Copy(X0) with accum_out -> sum(sel) per partition
"""

import sys
import types

for _p in ("/opt/trn_rl_repo",):
    if _p not in sys.path:
        sys.path.insert(0, _p)

import numpy as np


def _install_ntff_hook_shim():
    """This image's antenv lacks axon_hooks, so trn_boot's NTFF profile hook
    registration degrades silently and bass_utils crashes on import if tracing
    is requested (e.g. BASS_TRACE=1).  Recreate the module and register the
    ctypes hook trn_boot would have installed.  No-op if the module exists."""
    try:
        import antenv.axon_hooks  # noqa: F401

        return
    except ImportError:
        pass
    try:
        import antenv
        from trn_agent_boot.trn_boot import _ntff_profile_via_ctypes
    except ImportError:
        return
    mod = types.ModuleType("antenv.axon_hooks")
    mod._hook = None
    mod.set_axon_ntff_profile_hook = lambda h: setattr(mod, "_hook", h)
    mod.get_axon_ntff_profile_hook = lambda: mod._hook
    sys.modules["antenv.axon_hooks"] = mod
    antenv.axon_hooks = mod
    try:
        mod._hook = _ntff_profile_via_ctypes("/opt/axon/libaxon_pjrt.so")
    except Exception:
        pass


_install_ntff_hook_shim()

import concourse.bacc as bacc
import concourse.bass as bass
import concourse.mybir as mybir
from concourse.bass_utils import run_bass_kernel_spmd
from concourse.mybir import AluOpType
from concourse.tile import TileContext

AF = mybir.ActivationFunctionType
F32 = mybir.dt.float32
U8 = mybir.dt.uint8
I8 = mybir.dt.int8

P = 128
B, N = 64, 65536
M = 8                      # cores
BS = B // M                # batches per core
NP_CORE = BS * N           # points per core
T = 4                      # tiles per core
L = NP_CORE // (T * P)     # points per partition per tile


def emit_loss_kernel(
    nc, conf, lg, tgt, out_acc, n_tiles, width, io_bufs=2, tmp_bufs=2, repeat=1,
    ops=frozenset({"exp", "sums", "pm", "sel", "ln", "selsum"}),
):
    """Emit the per-core loss kernel.

    conf: DRAM AP [T, 128, 4L]  interleaved 4-class confusion logits
    lg:   DRAM AP [T, 128, 2L]  interleaved 2-class prediction logits
    tgt:  DRAM AP [T, 128, L]   target class as uint8 0/1
    out_acc: DRAM AP [128, 2T]  cols [0,T) = sum(lse), cols [T,2T) = sum(sel)
    repeat: re-run the whole pass this many times (benchmark differencing)
    """
    with TileContext(nc) as tc:
        with (
            tc.tile_pool(name="io", bufs=io_bufs) as io_pool,
            tc.tile_pool(name="tmp", bufs=tmp_bufs) as tmp_pool,
            tc.tile_pool(name="accp", bufs=1) as acc_pool,
        ):
            acc = acc_pool.tile([P, 2 * n_tiles], F32)
            if not ({"ln", "selsum"} & ops):
                nc.vector.memset(acc[:], 0.0)
            for _rep, t in ((r, t) for r in range(repeat) for t in range(n_tiles)):
                conf_t = io_pool.tile([P, 4 * width], F32, tag="conf")
                lg_t = io_pool.tile([P, 2 * width], F32, tag="lg")
                tgt_t = io_pool.tile([P, width], U8, tag="tgt")
                nc.sync.dma_start(out=conf_t[:], in_=conf[t])
                nc.sync.dma_start(out=lg_t[:], in_=lg[t])
                nc.sync.dma_start(out=tgt_t[:], in_=tgt[t])

                e_t = tmp_pool.tile([P, 4 * width], F32, tag="e")
                t01 = tmp_pool.tile([P, width], F32, tag="t01")
                t23 = tmp_pool.tile([P, width], F32, tag="t23")
                s = tmp_pool.tile([P, width], F32, tag="s")
                pm = tmp_pool.tile([P, width], I8, tag="pm")

                X = conf_t[:].rearrange("p (l k) -> p l k", k=4)
                E = e_t[:].rearrange("p (l k) -> p l k", k=4)
                LG = lg_t[:].rearrange("p (l k) -> p l k", k=2)

                if "exp" in ops:
                    nc.scalar.activation(e_t[:], conf_t[:], AF.Exp)
                if "sums" in ops:
                    nc.vector.tensor_tensor(t01[:], E[:, :, 0], E[:, :, 1], AluOpType.add)
                    nc.vector.tensor_tensor(t23[:], E[:, :, 2], E[:, :, 3], AluOpType.add)
                    nc.vector.tensor_tensor(s[:], t01[:], t23[:], AluOpType.add)
                if "pm" in ops:
                    nc.vector.tensor_tensor(pm[:], LG[:, :, 1], LG[:, :, 0], AluOpType.is_gt)
                if "sel" in ops:
                    nc.vector.copy_predicated(X[:, :, 0], pm[:], X[:, :, 3])
                    nc.vector.copy_predicated(X[:, :, 2], pm[:], X[:, :, 1])
                    nc.vector.copy_predicated(X[:, :, 0], tgt_t[:], X[:, :, 2])
                if "ln" in ops:
                    nc.scalar.activation(
                        t01[:], s[:], AF.Ln, accum_out=acc[:, t : t + 1]
                    )
                if "selsum" in ops:
                    nc.scalar.activation(
                        t23[:], X[:, :, 0], AF.Copy,
                        accum_out=acc[:, n_tiles + t : n_tiles + t + 1],
                    )
            nc.sync.dma_start(out=out_acc, in_=acc[:])
    return nc


def _pin_act_table_set(nc, set_id):
    """Replace the alternating per-function ACT table loads with a single
    load of one set that contains every function the kernel uses.

    bacc's insert_act_table_loads greedily picks the first act_info.json set
    containing each function, so an Exp/Ln/Copy mix thrashes between
    exp_and_others and natural_log -- ~2.7us per reload, serialized on ACT.
    natural_log_exp_and_others (set 6 on gen3) holds Exp, Ln and Copy, so one
    load suffices.  The inserted loads carry no sync_info, so dropping the
    extras cannot break semaphore bookkeeping.
    """
    for fn in nc.m.functions:
        for blk in fn.blocks:
            first = True
            keep = []
            for ins in blk.instructions:
                if isinstance(ins, mybir.InstLoadActFuncSet):
                    assert ins.sync_info is None or (
                        not ins.sync_info.on_wait and not ins.sync_info.on_update
                    )
                    if not first:
                        continue
                    ins.act_func_set_id = set_id
                    first = False
                keep.append(ins)
            if len(keep) != len(blk.instructions):
                blk.instructions[:] = keep


def build_nc(n_tiles=T, width=L, io_bufs=2, tmp_bufs=2, repeat=1,
             ops=frozenset({"exp", "sums", "pm", "sel", "ln", "selsum"})):
    nc = bacc.Bacc("TRN2", target_bir_lowering=False, debug=False)
    conf = nc.dram_tensor("conf", [n_tiles, P, 4 * width], F32, kind="ExternalInput").ap()
    lg = nc.dram_tensor("lg", [n_tiles, P, 2 * width], F32, kind="ExternalInput").ap()
    tgt = nc.dram_tensor("tgt", [n_tiles, P, width], U8, kind="ExternalInput").ap()
    out_acc = nc.dram_tensor("acc", [P, 2 * n_tiles], F32, kind="ExternalOutput").ap()
    emit_loss_kernel(
        nc, conf, lg, tgt, out_acc, n_tiles, width, io_bufs, tmp_bufs, repeat, ops
    )
    nc.finalize()
    _pin_act_table_set(nc, 6)
    return nc


BF16 = mybir.dt.bfloat16
I16 = mybir.dt.int16


def emit_loss_kernel_v2(
    nc, conf, lg, tgt, out_acc, n_tiles, width, io_bufs=3, tmp_bufs=2, repeat=1,
    ops=frozenset({"exp", "sums", "pm", "sel", "ln", "selsum"}),
):
    """Planar bf16 variant.

    conf: DRAM AP [T, 128, 4L] bf16, free dim = [class k][point l] (planar)
    lg:   DRAM AP [T, 128, 2L] bf16 planar
    tgt:  DRAM AP [T, 128, L]  uint8
    out_acc: DRAM AP [128, T+1]: cols [0,T) per-partition sum(lse); entry
      [0, T] = total sum(sel) (from the PE column-sum path, partition 0 only).

    All DVE ops are step-1 16-bit -> 2x_1P mode.  sum(sel) is computed by the
    TensorEngine as a ones-vector column sum accumulated in PSUM across tiles,
    then one ACT Copy+accum over [1, L] at the end.
    """
    W = width
    with TileContext(nc) as tc:
        with (
            tc.tile_pool(name="io", bufs=io_bufs) as io_pool,
            tc.tile_pool(name="tmp", bufs=tmp_bufs) as tmp_pool,
            tc.tile_pool(name="accp", bufs=1) as acc_pool,
            tc.tile_pool(name="psum", bufs=1, space="PSUM") as psum_pool,
        ):
            acc = acc_pool.tile([P, 2 * n_tiles + 1], F32)
            ones = acc_pool.tile([P, 1], BF16)
            nc.vector.memset(ones[:], 1.0)
            nc.vector.memset(acc[:], 0.0)
            selp = psum_pool.tile([1, W], F32)
            n_chunks = (W + 511) // 512
            total = repeat * n_tiles
            it = 0
            for _rep, t in ((r, t) for r in range(repeat) for t in range(n_tiles)):
                conf_t = io_pool.tile([P, 4 * W], BF16, tag="conf")
                lg_t = io_pool.tile([P, 2 * W], BF16, tag="lg")
                tgt_t = io_pool.tile([P, W], U8, tag="tgt")
                nc.sync.dma_start(out=conf_t[:], in_=conf[t])
                nc.sync.dma_start(out=lg_t[:], in_=lg[t])
                nc.sync.dma_start(out=tgt_t[:], in_=tgt[t])

                e_t = tmp_pool.tile([P, 4 * W], BF16, tag="e")
                s01 = tmp_pool.tile([P, W], BF16, tag="s01")
                s23 = tmp_pool.tile([P, W], BF16, tag="s23")
                s = tmp_pool.tile([P, W], BF16, tag="s")
                pm = tmp_pool.tile([P, W], I16, tag="pm")

                ca = conf_t[:]
                ea = e_t[:]
                la = lg_t[:]
                Xs = [ca[:, k * W : (k + 1) * W] for k in range(4)]
                Es = [ea[:, k * W : (k + 1) * W] for k in range(4)]
                L0 = la[:, 0:W]
                L1 = la[:, W : 2 * W]

                if "pm" in ops:
                    nc.vector.tensor_tensor(pm[:], L1, L0, AluOpType.is_gt)
                if "exp" in ops:
                    nc.scalar.activation(e_t[:], conf_t[:], AF.Exp)
                if "sums" in ops:
                    eng01 = nc.gpsimd if "pooladds" in ops else nc.vector
                    eng01.tensor_tensor(s01[:], Es[0], Es[1], AluOpType.add)
                    eng01.tensor_tensor(s23[:], Es[2], Es[3], AluOpType.add)
                    nc.vector.tensor_tensor(s[:], s01[:], s23[:], AluOpType.add)
                if "sel" in ops:
                    nc.vector.copy_predicated(Xs[0], pm[:], Xs[3])
                    nc.vector.copy_predicated(Xs[2], pm[:], Xs[1])
                    nc.vector.copy_predicated(Xs[0], tgt_t[:], Xs[2])
                if "ln" in ops:
                    nc.scalar.activation(
                        s01[:], s[:], AF.Ln, accum_out=acc[:, t : t + 1]
                    )
                if "selsum" in ops:
                    if "selsum_pe" in ops:
                        for c in range(n_chunks):
                            lo, hi = c * 512, min((c + 1) * 512, W)
                            nc.tensor.matmul(
                                selp[:, lo:hi],
                                ones[:],
                                Xs[0][:, lo:hi],
                                start=(it == 0),
                                stop=(it == total - 1),
                            )
                    else:
                        nc.scalar.activation(
                            s23[:], Xs[0], AF.Copy,
                            accum_out=acc[:, n_tiles + 1 + t : n_tiles + 2 + t],
                        )
                it += 1
            if "selsum" in ops and "selsum_pe" in ops:
                seljunk = acc_pool.tile([1, W], F32)
                nc.scalar.activation(
                    seljunk[:],
                    selp[:, :],
                    AF.Copy,
                    accum_out=acc[0:1, n_tiles : n_tiles + 1],
                )
            nc.sync.dma_start(out=out_acc, in_=acc[:])
    return nc


def build_nc_v2(n_tiles=T, width=None, io_bufs=3, tmp_bufs=2, repeat=1,
                ops=frozenset({"exp", "sums", "pm", "sel", "ln", "selsum"})):
    if width is None:
        width = NP_CORE // (n_tiles * P)
    nc = bacc.Bacc("TRN2", target_bir_lowering=False, debug=False)
    conf = nc.dram_tensor("conf", [n_tiles, P, 4 * width], BF16, kind="ExternalInput").ap()
    lg = nc.dram_tensor("lg", [n_tiles, P, 2 * width], BF16, kind="ExternalInput").ap()
    tgt = nc.dram_tensor("tgt", [n_tiles, P, width], U8, kind="ExternalInput").ap()
    out_acc = nc.dram_tensor("acc", [P, 2 * n_tiles + 1], F32, kind="ExternalOutput").ap()
    emit_loss_kernel_v2(
        nc, conf, lg, tgt, out_acc, n_tiles, width, io_bufs, tmp_bufs, repeat, ops
    )
    nc.finalize()
    _pin_act_table_set(nc, 6)
    return nc


def shard_inputs_v2(pred_logits, pred_confusion, target_classes, n_tiles=T, width=None):
    import ml_dtypes

    if width is None:
        width = NP_CORE // (n_tiles * P)
    T_, L_ = n_tiles, width
    bf16 = ml_dtypes.bfloat16
    in_maps = []
    for i in range(M):
        sl = slice(i * BS, (i + 1) * BS)
        conf = (
            np.asarray(pred_confusion[sl], dtype=np.float32)
            .reshape(T_, P, L_, 4)
            .transpose(0, 1, 3, 2)
            .astype(bf16)
            .reshape(T_, P, 4 * L_)
        )
        conf = np.ascontiguousarray(conf)
        lg = (
            np.asarray(pred_logits[sl], dtype=np.float32)
            .reshape(T_, P, L_, 2)
            .transpose(0, 1, 3, 2)
            .astype(bf16)
            .reshape(T_, P, 2 * L_)
        )
        lg = np.ascontiguousarray(lg)
        tgt = np.asarray(target_classes[sl], dtype=np.uint8).reshape(T_, P, L_)
        in_maps.append({"conf": conf, "lg": lg, "tgt": tgt})
    return in_maps


def shard_inputs(pred_logits, pred_confusion, target_classes):
    in_maps = []
    for i in range(M):
        sl = slice(i * BS, (i + 1) * BS)
        conf = np.ascontiguousarray(pred_confusion[sl], dtype=np.float32).reshape(
            T, P, 4 * L
        )
        lg = np.ascontiguousarray(pred_logits[sl], dtype=np.float32).reshape(
            T, P, 2 * L
        )
        tgt = np.asarray(target_classes[sl], dtype=np.uint8).reshape(T, P, L)
        in_maps.append({"conf": conf, "lg": lg, "tgt": tgt})
    return in_maps


FP8 = mybir.dt.float8e4


def emit_loss_kernel_v3(
    nc, conf, lg, tgt, out_acc, n_tiles, width, io_bufs=3, tmp_bufs=2, repeat=1,
    pool_s=True,
):
    """fp8 planar variant with minimal DVE work.

    conf: DRAM AP [T, 128, 4W] fp8, planes ordered [x2 | x0 | x1 | x3]
    lg:   DRAM AP [T, 128, 2W] bf16, planes [l0 | l1]
    tgt:  DRAM AP [T, 128, W]  uint8 (0/1)
    out_acc: DRAM AP [128, T+1]: cols [0,T) per-partition sum(lse(tile));
      entry [0, T] = total sum(selected logit) via the PE column-sum path.

    Engine split per tile:
      ACT : E = exp(conf) [4W], Ln(s)+accum [W]
      DVE : s2 = E[:2W]+E[2W:], pm = l1>l0, 3 copy_predicated (selection)
      Pool: s = s2[:W]+s2[W:]
      PE  : psum += ones^T @ sel  (column sums, accumulated across tiles)
    """
    W = width
    with TileContext(nc) as tc:
        with (
            tc.tile_pool(name="io", bufs=io_bufs) as io_pool,
            tc.tile_pool(name="tmp", bufs=tmp_bufs) as tmp_pool,
            tc.tile_pool(name="accp", bufs=1) as acc_pool,
            tc.tile_pool(name="psum", bufs=1, space="PSUM") as psum_pool,
        ):
            acc = acc_pool.tile([P, n_tiles + 1], F32)
            ones = acc_pool.tile([P, 1], BF16)
            nc.vector.memset(ones[:], 1.0)
            nc.vector.memset(acc[:], 0.0)
            selp = psum_pool.tile([1, W], F32)
            n_chunks = (W + 511) // 512
            total = repeat * n_tiles
            it = 0
            for _rep, t in ((r, t) for r in range(repeat) for t in range(n_tiles)):
                conf_t = io_pool.tile([P, 4 * W], FP8, tag="conf")
                lg_t = io_pool.tile([P, 2 * W], BF16, tag="lg")
                tgt_t = io_pool.tile([P, W], U8, tag="tgt")
                nc.sync.dma_start(out=conf_t[:], in_=conf[t])
                nc.sync.dma_start(out=lg_t[:], in_=lg[t])
                nc.sync.dma_start(out=tgt_t[:], in_=tgt[t])

                e_t = tmp_pool.tile([P, 4 * W], BF16, tag="e")
                s2 = tmp_pool.tile([P, 2 * W], BF16, tag="s2")
                s = tmp_pool.tile([P, W], BF16, tag="s")
                pm = tmp_pool.tile([P, W], I16, tag="pm")

                ca = conf_t[:]
                A = ca[:, : 2 * W]          # [x2 | x0]
                B = ca[:, 2 * W :]          # [x1 | x3]
                ea = e_t[:]

                nc.scalar.activation(e_t[:], conf_t[:], AF.Exp)
                nc.vector.tensor_tensor(
                    s2[:], ea[:, : 2 * W], ea[:, 2 * W :], AluOpType.add
                )
                eng_s = nc.gpsimd if pool_s else nc.vector
                eng_s.tensor_tensor(s[:], s2[:, :W], s2[:, W:], AluOpType.add)
                nc.vector.tensor_tensor(pm[:], lg_t[:, W:], lg_t[:, :W], AluOpType.is_gt)
                # selection: A[:, :W] = pm ? x1 : x2 ; A[:, W:] = pm ? x3 : x0
                nc.vector.copy_predicated(A[:, :W], pm[:], B[:, :W])
                nc.vector.copy_predicated(A[:, W:], pm[:], B[:, W:])
                # g-select: sel = g ? A[:, :W] : A[:, W:]  ->  A[:, W:]
                nc.vector.copy_predicated(A[:, W:], tgt_t[:], A[:, :W])
                nc.scalar.activation(
                    s2[:, :W], s[:], AF.Ln, accum_out=acc[:, t : t + 1]
                )
                for c in range(n_chunks):
                    lo, hi = c * 512, min((c + 1) * 512, W)
                    nc.tensor.matmul(
                        selp[:, lo:hi],
                        ones[:],
                        A[:, W + lo : W + hi],
                        start=(it == 0),
                        stop=(it == total - 1),
                    )
                it += 1
            seljunk = acc_pool.tile([1, W], F32)
            nc.scalar.activation(
                seljunk[:],
                selp[:, :],
                AF.Copy,
                accum_out=acc[0:1, n_tiles : n_tiles + 1],
            )
            nc.sync.dma_start(out=out_acc, in_=acc[:])
    return nc


def build_nc_v3(n_tiles=T, width=None, io_bufs=3, tmp_bufs=2, repeat=1, pool_s=True):
    if width is None:
        width = NP_CORE // (n_tiles * P)
    nc = bacc.Bacc("TRN2", target_bir_lowering=False, debug=False)
    conf = nc.dram_tensor("conf", [n_tiles, P, 4 * width], FP8, kind="ExternalInput").ap()
    lg = nc.dram_tensor("lg", [n_tiles, P, 2 * width], BF16, kind="ExternalInput").ap()
    tgt = nc.dram_tensor("tgt", [n_tiles, P, width], U8, kind="ExternalInput").ap()
    out_acc = nc.dram_tensor("acc", [P, n_tiles + 1], F32, kind="ExternalOutput").ap()
    emit_loss_kernel_v3(
        nc, conf, lg, tgt, out_acc, n_tiles, width, io_bufs, tmp_bufs, repeat, pool_s
    )
    nc.finalize()
    _pin_act_table_set(nc, 6)
    return nc


def shard_inputs_v3(pred_logits, pred_confusion, target_classes, n_tiles=T, width=None):
    import ml_dtypes

    if width is None:
        width = NP_CORE // (n_tiles * P)
    T_, L_ = n_tiles, width
    bf16 = ml_dtypes.bfloat16
    fp8 = ml_dtypes.float8_e4m3
    in_maps = []
    for i in range(M):
        sl = slice(i * BS, (i + 1) * BS)
        conf = (
            np.asarray(pred_confusion[sl], dtype=np.float32)
            .reshape(T_, P, L_, 4)
            .transpose(0, 1, 3, 2)[:, :, [2, 0, 1, 3], :]
            .astype(fp8)
            .reshape(T_, P, 4 * L_)
        )
        conf = np.ascontiguousarray(conf)
        lg = (
            np.asarray(pred_logits[sl], dtype=np.float32)
            .reshape(T_, P, L_, 2)
            .transpose(0, 1, 3, 2)
            .astype(bf16)
            .reshape(T_, P, 2 * L_)
        )
        lg = np.ascontiguousarray(lg)
        tgt = np.asarray(target_classes[sl], dtype=np.uint8).reshape(T_, P, L_)
        in_maps.append({"conf": conf, "lg": lg, "tgt": tgt})
    return in_maps


def reduce_v3(results, n_tiles=T):
    total = 0.0
    for r in results:
        a = np.asarray(r["acc"], dtype=np.float64)
        total += a[:, :n_tiles].sum() - a[0, n_tiles]
    return np.float32(total / (B * N))


def kernel_v3(pred_logits, pred_confusion, target_classes):
    if "nc3" not in _CACHED:
        _CACHED["nc3"] = build_nc_v3()
    in_maps = shard_inputs_v3(pred_logits, pred_confusion, target_classes)
    results = run_bass_kernel_spmd(_CACHED["nc3"], in_maps, list(range(M))).results
    return reduce_v3(results)


def emit_loss_kernel_v4(
    nc, conf, lg, tgt, out_acc, n_tiles, width, io_bufs=3, tmp_bufs=2,
    repeat=1,
):
    """Pair-interleaved fp8 variant.

    conf: DRAM AP [T, 128, 4W] fp8. Free layout per tile:
      bytes [0, 2W):  A-half, interleaved pairs (x2[i], x0[i])
      bytes [2W, 4W): B-half, interleaved pairs (x1[i], x3[i])
    lg:  DRAM AP [T, 128, 2W] bf16 planar [l0 | l1]
    tgt: DRAM AP [T, 128, W] uint8
    out_acc: [128, T+1] f32: cols [0,T) per-partition per-tile sum(ln(s));
    entry [0, T] = total sum of the selected logit

    Per tile: ACT exp (de-interleaving output AP) + Ln/accum; DVE does one
    2W add, one W add, is_gt, an int16 copy_predicated (moves both
    pm-selected fp8 bytes per point under one mask element) and a strided
    fp8 copy_predicated for the g-select; PE accumulates column sums of the
    selected values in PSUM.
    """
    W = width
    I16_ = I16
    with TileContext(nc) as tc:
        with (
            tc.tile_pool(name="io", bufs=io_bufs) as io_pool,
            tc.tile_pool(name="tmp", bufs=tmp_bufs) as tmp_pool,
            tc.tile_pool(name="accp", bufs=1) as acc_pool,
            tc.tile_pool(name="psum", bufs=1, space="PSUM") as psum_pool,
        ):
            acc = acc_pool.tile([P, n_tiles + 1], F32)
            ones = acc_pool.tile([P, 1], BF16)
            nc.vector.memset(ones[:], 1.0)
            nc.vector.memset(acc[:], 0.0)
            selp = psum_pool.tile([1, W], F32)
            n_chunks = (W + 511) // 512
            total = repeat * n_tiles
            it = 0
            for _rep, t in ((r, t) for r in range(repeat) for t in range(n_tiles)):
                conf_t = io_pool.tile([P, 4 * W], FP8, tag="conf")
                lg_t = io_pool.tile([P, 2 * W], BF16, tag="lg")
                tgt_t = io_pool.tile([P, W], U8, tag="tgt")
                nc.sync.dma_start(out=conf_t[:], in_=conf[t])
                nc.sync.dma_start(out=lg_t[:], in_=lg[t])
                nc.sync.dma_start(out=tgt_t[:], in_=tgt[t])

                e_t = tmp_pool.tile([P, 4 * W], BF16, tag="e")
                s2 = tmp_pool.tile([P, 2 * W], BF16, tag="s2")
                s = tmp_pool.tile([P, W], BF16, tag="s")
                pm = tmp_pool.tile([P, W], I16_, tag="pm")

                ca = conf_t[:]
                ea = e_t[:]
                # exp, de-interleaving via strided READS (free on ACT; strided
                # writes are catastrophically slow): input addr 2W*h + 2w + k
                # -> output addr 2W*h + W*k + w, iterated (h, k, w).
                cin = ca.rearrange("p (h w k) -> p h k w", h=2, k=2)
                eout = ea.rearrange("p (h k w) -> p h k w", h=2, k=2)
                nc.scalar.activation(eout, cin, AF.Exp)
                # e_t now planar: [e2 | e0 | e1 | e3]
                nc.vector.tensor_tensor(
                    s2[:], ea[:, : 2 * W], ea[:, 2 * W :], AluOpType.add
                )  # [e2+e1 | e0+e3]
                nc.vector.tensor_tensor(s[:], s2[:, :W], s2[:, W:], AluOpType.add)
                nc.vector.tensor_tensor(
                    pm[:], lg_t[:, W:], lg_t[:, :W], AluOpType.is_gt
                )
                # pm-select both branches at once on the int16 view:
                # A-pairs (x2,x0) <- B-pairs (x1,x3) where pm
                c16 = ca.bitcast(I16_)
                nc.vector.copy_predicated(c16[:, :W], pm[:], c16[:, W:])
                # g-select: odd bytes (x0 slot) <- even bytes (x2 slot) where g
                apairs = ca[:, : 2 * W].rearrange("p (w k) -> p k w", k=2)
                sel_ap = apairs[:, 1]
                nc.vector.copy_predicated(sel_ap, tgt_t[:], apairs[:, 0])
                nc.scalar.activation(
                    s2[:, :W], s[:], AF.Ln, accum_out=acc[:, t : t + 1]
                )
                for c in range(n_chunks):
                    lo, hi = c * 512, min((c + 1) * 512, W)
                    nc.tensor.matmul(
                        selp[:, lo:hi],
                        ones[:],
                        sel_ap[:, lo:hi],
                        start=(it == 0),
                        stop=(it == total - 1),
                    )
                it += 1
            seljunk = acc_pool.tile([1, W], F32)
            nc.scalar.activation(
                seljunk[:], selp[:, :], AF.Copy,
                accum_out=acc[0:1, n_tiles : n_tiles + 1],
            )
            nc.sync.dma_start(out=out_acc, in_=acc[:])
    return nc


def build_nc_v4(n_tiles=T, width=None, io_bufs=3, tmp_bufs=2, repeat=1):
    if width is None:
        width = NP_CORE // (n_tiles * P)
    nc = bacc.Bacc("TRN2", target_bir_lowering=False, debug=False)
    conf = nc.dram_tensor("conf", [n_tiles, P, 4 * width], FP8, kind="ExternalInput").ap()
    lg = nc.dram_tensor("lg", [n_tiles, P, 2 * width], BF16, kind="ExternalInput").ap()
    tgt = nc.dram_tensor("tgt", [n_tiles, P, width], U8, kind="ExternalInput").ap()
    out_acc = nc.dram_tensor("acc", [P, n_tiles + 1], F32, kind="ExternalOutput").ap()
    emit_loss_kernel_v4(
        nc, conf, lg, tgt, out_acc, n_tiles, width, io_bufs, tmp_bufs, repeat
    )
    nc.finalize()
    _pin_act_table_set(nc, 6)
    return nc


def shard_inputs_v4(pred_logits, pred_confusion, target_classes, n_tiles=T, width=None):
    import ml_dtypes

    if width is None:
        width = NP_CORE // (n_tiles * P)
    T_, L_ = n_tiles, width
    bf16 = ml_dtypes.bfloat16
    fp8 = ml_dtypes.float8_e4m3
    in_maps = []
    for i in range(M):
        sl = slice(i * BS, (i + 1) * BS)
        c = np.asarray(pred_confusion[sl], dtype=np.float32).reshape(T_, P, L_, 4)
        conf = np.empty((T_, P, 4 * L_), dtype=fp8)
        ch = conf.reshape(T_, P, 2, L_, 2)
        ch[:, :, 0, :, 0] = c[..., 2]
        ch[:, :, 0, :, 1] = c[..., 0]
        ch[:, :, 1, :, 0] = c[..., 1]
        ch[:, :, 1, :, 1] = c[..., 3]
        lg = (
            np.asarray(pred_logits[sl], dtype=np.float32)
            .reshape(T_, P, L_, 2)
            .transpose(0, 1, 3, 2)
            .astype(bf16)
            .reshape(T_, P, 2 * L_)
        )
        lg = np.ascontiguousarray(lg)
        tgt = np.asarray(target_classes[sl], dtype=np.uint8).reshape(T_, P, L_)
        in_maps.append({"conf": conf, "lg": lg, "tgt": tgt})
    return in_maps


def reduce_v4(results, n_tiles=T):
    total = 0.0
    for r in results:
        a = np.asarray(r["acc"], dtype=np.float64)
        total += a[:, :n_tiles].sum() - a[0, n_tiles]
    return np.float32(total / (B * N))


def kernel_v4(pred_logits, pred_confusion, target_classes):
    if "nc4" not in _CACHED:
        _CACHED["nc4"] = build_nc_v4()
    in_maps = shard_inputs_v4(pred_logits, pred_confusion, target_classes)
    results = run_bass_kernel_spmd(_CACHED["nc4"], in_maps, list(range(M))).results
    return reduce_v4(results)


V5_WIDTHS = (512, 1024, 1024, 1024, 512)


def emit_loss_kernel_v5(nc, conf, lg, tgt, out_acc, widths, io_bufs=4, tmp_bufs=2):
    """v4 pipeline with nonuniform tile widths (small first tile so ACT can
    start sooner; small last tile so the tail chain is short).

    conf: DRAM AP [P, 4*sum(W)] fp8; tile t occupies columns
      [4*off_t, 4*off_t + 4*W_t), laid out [A-half pairs | B-half pairs]
    lg:   DRAM AP [P, 2*sum(W)] bf16; tile block [l0 plane | l1 plane]
    tgt:  DRAM AP [P, sum(W)] u8
    out_acc: [P, T+1] f32 (cols 0..T-1: per-tile ln sums; [0,T]: sel sum)
    """
    T_ = len(widths)
    offs = [0]
    for w in widths:
        offs.append(offs[-1] + w)
    Wmax = max(widths)
    with TileContext(nc) as tc:
        with (
            tc.tile_pool(name="io", bufs=io_bufs) as io_pool,
            tc.tile_pool(name="tmp", bufs=tmp_bufs) as tmp_pool,
            tc.tile_pool(name="accp", bufs=1) as acc_pool,
            tc.tile_pool(name="psum", bufs=1, space="PSUM") as psum_pool,
        ):
            acc = acc_pool.tile([P, T_ + 1], F32)
            ones = acc_pool.tile([P, 1], BF16)
            nc.vector.memset(ones[:], 1.0)
            nc.vector.memset(acc[:], 0.0)
            selp = psum_pool.tile([1, 512], F32)
            n_chunk_total = sum((w + 511) // 512 for w in widths)
            ic = 0
            for t, W in enumerate(widths):
                off = offs[t]
                conf_t = io_pool.tile([P, 4 * Wmax], FP8, tag="conf")
                lg_t = io_pool.tile([P, 2 * Wmax], BF16, tag="lg")
                tgt_t = io_pool.tile([P, Wmax], U8, tag="tgt")
                nc.sync.dma_start(
                    out=conf_t[:, : 4 * W], in_=conf[:, 4 * off : 4 * (off + W)]
                )
                nc.sync.dma_start(
                    out=lg_t[:, : 2 * W], in_=lg[:, 2 * off : 2 * (off + W)]
                )
                nc.sync.dma_start(out=tgt_t[:, :W], in_=tgt[:, off : off + W])

                e_t = tmp_pool.tile([P, 4 * Wmax], BF16, tag="e")
                s2 = tmp_pool.tile([P, 2 * Wmax], BF16, tag="s2")
                s = tmp_pool.tile([P, Wmax], BF16, tag="s")
                pm = tmp_pool.tile([P, Wmax], I16, tag="pm")

                ca = conf_t[:, : 4 * W]
                ea = e_t[:, : 4 * W]
                cin = ca.rearrange("p (h w k) -> p h k w", h=2, k=2)
                eout = ea.rearrange("p (h k w) -> p h k w", h=2, k=2)
                nc.scalar.activation(eout, cin, AF.Exp)
                nc.vector.tensor_tensor(
                    s2[:, : 2 * W], ea[:, : 2 * W], ea[:, 2 * W :], AluOpType.add
                )
                nc.vector.tensor_tensor(
                    s[:, :W], s2[:, :W], s2[:, W : 2 * W], AluOpType.add
                )
                nc.vector.tensor_tensor(
                    pm[:, :W], lg_t[:, W : 2 * W], lg_t[:, :W], AluOpType.is_gt
                )
                c16 = conf_t[:].bitcast(I16)
                nc.vector.copy_predicated(c16[:, :W], pm[:, :W], c16[:, W : 2 * W])
                apairs = ca[:, : 2 * W].rearrange("p (w k) -> p k w", k=2)
                sel_ap = apairs[:, 1]
                nc.vector.copy_predicated(sel_ap, tgt_t[:, :W], apairs[:, 0])
                nc.scalar.activation(
                    s2[:, :W], s[:, :W], AF.Ln, accum_out=acc[:, t : t + 1]
                )
                for c in range((W + 511) // 512):
                    lo, hi = c * 512, min((c + 1) * 512, W)
                    nc.tensor.matmul(
                        selp[:, : hi - lo],
                        ones[:],
                        sel_ap[:, lo:hi],
                        start=(ic == 0),
                        stop=(ic == n_chunk_total - 1),
                    )
                    ic += 1
            seljunk = acc_pool.tile([1, 512], F32)
            nc.scalar.activation(
                seljunk[:], selp[:, :], AF.Copy,
                accum_out=acc[0:1, T_ : T_ + 1],
            )
            nc.sync.dma_start(out=out_acc, in_=acc[:])
    return nc


def build_nc_v5(widths=V5_WIDTHS, io_bufs=4, tmp_bufs=2):
    total = sum(widths)
    assert total * P == NP_CORE
    nc = bacc.Bacc("TRN2", target_bir_lowering=False, debug=False)
    conf = nc.dram_tensor("conf", [P, 4 * total], FP8, kind="ExternalInput").ap()
    lg = nc.dram_tensor("lg", [P, 2 * total], BF16, kind="ExternalInput").ap()
    tgt = nc.dram_tensor("tgt", [P, total], U8, kind="ExternalInput").ap()
    out_acc = nc.dram_tensor(
        "acc", [P, len(widths) + 1], F32, kind="ExternalOutput"
    ).ap()
    emit_loss_kernel_v5(nc, conf, lg, tgt, out_acc, widths, io_bufs, tmp_bufs)
    nc.finalize()
    _pin_act_table_set(nc, 6)
    return nc


def shard_inputs_v5(pred_logits, pred_confusion, target_classes, widths=V5_WIDTHS):
    import ml_dtypes

    bf16 = ml_dtypes.bfloat16
    fp8 = ml_dtypes.float8_e4m3
    total = sum(widths)
    offs = [0]
    for w in widths:
        offs.append(offs[-1] + w)
    in_maps = []
    for i in range(M):
        sl = slice(i * BS, (i + 1) * BS)
        c = np.asarray(pred_confusion[sl], dtype=np.float32).reshape(P, total, 4)
        lgf = np.asarray(pred_logits[sl], dtype=np.float32).reshape(P, total, 2)
        tgf = np.asarray(target_classes[sl], dtype=np.uint8).reshape(P, total)
        conf = np.empty((P, 4 * total), dtype=fp8)
        lg = np.empty((P, 2 * total), dtype=bf16)
        for t, W in enumerate(widths):
            off = offs[t]
            cb = c[:, off : off + W, :]
            blk = conf[:, 4 * off : 4 * (off + W)].reshape(P, 2, W, 2)
            blk[:, 0, :, 0] = cb[..., 2]
            blk[:, 0, :, 1] = cb[..., 0]
            blk[:, 1, :, 0] = cb[..., 1]
            blk[:, 1, :, 1] = cb[..., 3]
            lb = lg[:, 2 * off : 2 * (off + W)].reshape(P, 2, W)
            lb[:, 0, :] = lgf[:, off : off + W, 0]
            lb[:, 1, :] = lgf[:, off : off + W, 1]
        in_maps.append({"conf": conf, "lg": lg, "tgt": tgf})
    return in_maps


def reduce_v5(results, widths=V5_WIDTHS):
    T_ = len(widths)
    total = 0.0
    for r in results:
        a = np.asarray(r["acc"], dtype=np.float64)
        total += a[:, :T_].sum() - a[0, T_]
    return np.float32(total / (B * N))


def kernel_v5(pred_logits, pred_confusion, target_classes):
    if "nc5" not in _CACHED:
        _CACHED["nc5"] = build_nc_v5()
    in_maps = shard_inputs_v5(pred_logits, pred_confusion, target_classes)
    results = run_bass_kernel_spmd(_CACHED["nc5"], in_maps, list(range(M))).results
    return reduce_v5(results)


def emit_loss_kernel_v6(
    nc, conf, lg, tgt, out_acc, n_tiles, width, io_bufs=3, tmp_bufs=2,
    prod_depth=1, split_t0=True,
):
    """v4 + schedule trims.

    - tile 0's conf DMA and exp are split into A/B halves so ACT starts as
      soon as half the first tile has landed
    - ln operates on pairwise products of s (prod_depth levels), shifting
      work from ACT (1x) to DVE (2x); Sum ln(s_i) == Sum ln(prod pairs)
    - the PSUM column-sum of the selected logits is exported with a DVE
      tensor_copy + DMA instead of an ACT copy (shorter tail)
    out_acc: [P, T+1] f32 (cols 0..T-1 ln sums; [0, T] total sel sum)
    """
    W = width
    with TileContext(nc) as tc:
        with (
            tc.tile_pool(name="io", bufs=io_bufs) as io_pool,
            tc.tile_pool(name="tmp", bufs=tmp_bufs) as tmp_pool,
            tc.tile_pool(name="accp", bufs=1) as acc_pool,
            tc.tile_pool(name="psum", bufs=1, space="PSUM") as psum_pool,
        ):
            acc = acc_pool.tile([P, n_tiles + 1], F32)
            ones = acc_pool.tile([P, 1], BF16)
            nc.vector.memset(ones[:], 1.0)
            nc.vector.memset(acc[:], 0.0)
            out_sel_acc = acc[0:1, n_tiles : n_tiles + 1]
            selp = psum_pool.tile([1, 512], F32)
            n_chunks = (W + 511) // 512
            for t in range(n_tiles):
                conf_t = io_pool.tile([P, 4 * W], FP8, tag="conf")
                lg_t = io_pool.tile([P, 2 * W], BF16, tag="lg")
                tgt_t = io_pool.tile([P, W], U8, tag="tgt")
                if t == 0 and split_t0:
                    nc.sync.dma_start(out=conf_t[:, : 2 * W], in_=conf[t][:, : 2 * W])
                    nc.sync.dma_start(out=conf_t[:, 2 * W :], in_=conf[t][:, 2 * W :])
                else:
                    nc.sync.dma_start(out=conf_t[:], in_=conf[t])
                nc.sync.dma_start(out=lg_t[:], in_=lg[t])
                nc.sync.dma_start(out=tgt_t[:], in_=tgt[t])

                e_t = tmp_pool.tile([P, 4 * W], BF16, tag="e")
                s2 = tmp_pool.tile([P, 2 * W], BF16, tag="s2")
                s = tmp_pool.tile([P, W], BF16, tag="s")
                pm = tmp_pool.tile([P, W], I16, tag="pm")

                ca = conf_t[:]
                ea = e_t[:]
                if t == 0 and split_t0:
                    for h in (0, 1):
                        cin = ca[:, 2 * W * h : 2 * W * (h + 1)].rearrange(
                            "p (w k) -> p k w", k=2
                        )
                        eout = ea[:, 2 * W * h : 2 * W * (h + 1)].rearrange(
                            "p (k w) -> p k w", k=2
                        )
                        nc.scalar.activation(eout, cin, AF.Exp)
                else:
                    cin = ca.rearrange("p (h w k) -> p h k w", h=2, k=2)
                    eout = ea.rearrange("p (h k w) -> p h k w", h=2, k=2)
                    nc.scalar.activation(eout, cin, AF.Exp)
                nc.vector.tensor_tensor(
                    s2[:], ea[:, : 2 * W], ea[:, 2 * W :], AluOpType.add
                )
                nc.vector.tensor_tensor(s[:], s2[:, :W], s2[:, W:], AluOpType.add)
                nc.vector.tensor_tensor(
                    pm[:], lg_t[:, W:], lg_t[:, :W], AluOpType.is_gt
                )
                c16 = ca.bitcast(I16)
                nc.vector.copy_predicated(c16[:, :W], pm[:], c16[:, W:])
                apairs = ca[:, : 2 * W].rearrange("p (w k) -> p k w", k=2)
                sel_ap = apairs[:, 1]
                nc.vector.copy_predicated(sel_ap, tgt_t[:], apairs[:, 0])
                # pairwise products: Sum ln(s) = Sum ln(prod of pairs)
                sp = tmp_pool.tile([P, W], BF16, tag="sp")
                lw = W
                bufs = (s, sp)
                for _d in range(prod_depth):
                    lw //= 2
                    src, dst = bufs[_d % 2][:], bufs[(_d + 1) % 2][:]
                    nc.vector.tensor_tensor(
                        dst[:, :lw], src[:, :lw], src[:, lw : 2 * lw],
                        AluOpType.mult,
                    )
                nc.scalar.activation(
                    s2[:, :lw], bufs[prod_depth % 2][:, :lw], AF.Ln,
                    accum_out=acc[:, t : t + 1],
                )
                for c in range(n_chunks):
                    lo, hi = c * 512, min((c + 1) * 512, W)
                    nc.tensor.matmul(
                        selp[:, : hi - lo],
                        ones[:],
                        sel_ap[:, lo:hi],
                        start=(t == 0 and c == 0),
                        stop=(t == n_tiles - 1 and c == n_chunks - 1),
                    )
            selsb = acc_pool.tile([1, 512], F32)
            nc.scalar.activation(selsb[:], selp[:], AF.Copy,
                                 accum_out=out_sel_acc)
            nc.sync.dma_start(out=out_acc, in_=acc[:])
    return nc


def build_nc_v6(n_tiles=T, width=None, io_bufs=3, tmp_bufs=2, prod_depth=1,
                split_t0=True):
    if width is None:
        width = NP_CORE // (n_tiles * P)
    nc = bacc.Bacc("TRN2", target_bir_lowering=False, debug=False)
    conf = nc.dram_tensor("conf", [n_tiles, P, 4 * width], FP8, kind="ExternalInput").ap()
    lg = nc.dram_tensor("lg", [n_tiles, P, 2 * width], BF16, kind="ExternalInput").ap()
    tgt = nc.dram_tensor("tgt", [n_tiles, P, width], U8, kind="ExternalInput").ap()
    out_acc = nc.dram_tensor("acc", [P, n_tiles + 1], F32, kind="ExternalOutput").ap()
    emit_loss_kernel_v6(
        nc, conf, lg, tgt, out_acc, n_tiles, width, io_bufs, tmp_bufs,
        prod_depth, split_t0,
    )
    nc.finalize()
    _pin_act_table_set(nc, 6)
    return nc


def reduce_v6(results, n_tiles=T):
    total = 0.0
    for r in results:
        a = np.asarray(r["acc"], dtype=np.float64)
        total += a[:, :n_tiles].sum() - a[0, n_tiles]
    return np.float32(total / (B * N))


def kernel_v6(pred_logits, pred_confusion, target_classes):
    if "nc6" not in _CACHED:
        _CACHED["nc6"] = build_nc_v6()
    in_maps = shard_inputs_v4(pred_logits, pred_confusion, target_classes)
    results = run_bass_kernel_spmd(_CACHED["nc6"], in_maps, list(range(M))).results
    return reduce_v6(results)


def emit_loss_kernel_v7(
    nc, conf, lg, tgt, out_acc, n_tiles, width, tmp_bufs=2, prod_depth=1,
    split_t0=True,
):
    """All-resident variant: the whole per-core input (36 KB/partition) is
    DMA'd up front into single SBUF tensors, in tile-priority order, so the
    DMA engines run flat out from the start and compute never recycles io
    buffers.  Per tile the DVE queue is ordered [sums, products] ->
    [selection] so the ACT Ln chain unblocks as early as possible.

    conf: DRAM [P, 4*T*W] fp8 (v5 flat layout: per-tile blocks [A|B] pairs)
    lg:   DRAM [P, 2*T*W] bf16 (per-tile blocks [l0|l1])
    tgt:  DRAM [P, T*W] u8
    out_acc: [P, T+1] f32 (cols 0..T-1 ln sums; [0,T] sel sum)
    """
    W = width
    T_ = n_tiles
    with TileContext(nc) as tc:
        with (
            tc.tile_pool(name="io", bufs=1) as io_pool,
            tc.tile_pool(name="tmp", bufs=tmp_bufs) as tmp_pool,
            tc.tile_pool(name="accp", bufs=1) as acc_pool,
            tc.tile_pool(name="psum", bufs=1, space="PSUM") as psum_pool,
        ):
            acc = acc_pool.tile([P, T_ + 1], F32)
            ones = acc_pool.tile([P, 1], BF16)
            nc.vector.memset(ones[:], 1.0)
            nc.vector.memset(acc[:], 0.0)
            selp = psum_pool.tile([1, 512], F32)
            conf_all = io_pool.tile([P, 4 * T_ * W], FP8, tag="conf")
            lg_all = io_pool.tile([P, 2 * T_ * W], BF16, tag="lg")
            tgt_all = io_pool.tile([P, T_ * W], U8, tag="tgt")

            # All input DMAs up front.  conf gates the ACT exp chain (the
            # critical engine), so conf transfers are interleaved ahead of
            # the lg/tgt streams, which only feed the trailing DVE selection.
            def dma_conf(t):
                c_sb = conf_all[:, 4 * W * t : 4 * W * (t + 1)]
                c_dr = conf[:, 4 * W * t : 4 * W * (t + 1)]
                if t == 0 and split_t0:
                    nc.sync.dma_start(out=c_sb[:, : 2 * W], in_=c_dr[:, : 2 * W])
                    nc.sync.dma_start(out=c_sb[:, 2 * W :], in_=c_dr[:, 2 * W :])
                else:
                    nc.sync.dma_start(out=c_sb, in_=c_dr)

            def dma_lg(t):
                nc.sync.dma_start(
                    out=lg_all[:, 2 * W * t : 2 * W * (t + 1)],
                    in_=lg[:, 2 * W * t : 2 * W * (t + 1)],
                )

            def dma_tgt(t):
                nc.sync.dma_start(
                    out=tgt_all[:, W * t : W * (t + 1)],
                    in_=tgt[:, W * t : W * (t + 1)],
                )

            rest = []
            for t in range(T_):
                rest += [("l", t), ("g", t)]
            seq = []
            j = 0
            for t in range(T_):
                seq.append(("c", t))
                if j < len(rest):
                    seq.append(rest[j])
                    j += 1
            seq += rest[j:]
            for kind, t in seq:
                (dma_conf if kind == "c" else dma_lg if kind == "l"
                 else dma_tgt)(t)
            n_chunks = (W + 511) // 512
            ic = 0
            for t in range(T_):
                ca = conf_all[:, 4 * W * t : 4 * W * (t + 1)]
                lga = lg_all[:, 2 * W * t : 2 * W * (t + 1)]
                tga = tgt_all[:, W * t : W * (t + 1)]
                e_t = tmp_pool.tile([P, 4 * W], BF16, tag="e")
                s2 = tmp_pool.tile([P, 2 * W], BF16, tag="s2")
                s = tmp_pool.tile([P, W], BF16, tag="s")
                sp = tmp_pool.tile([P, W], BF16, tag="sp")
                pm = tmp_pool.tile([P, W], I16, tag="pm")
                ea = e_t[:]
                if t == 0 and split_t0:
                    for h in (0, 1):
                        cin = ca[:, 2 * W * h : 2 * W * (h + 1)].rearrange(
                            "p (w k) -> p k w", k=2
                        )
                        eout = ea[:, 2 * W * h : 2 * W * (h + 1)].rearrange(
                            "p (k w) -> p k w", k=2
                        )
                        nc.scalar.activation(eout, cin, AF.Exp)
                else:
                    cin = ca.rearrange("p (h w k) -> p h k w", h=2, k=2)
                    eout = ea.rearrange("p (h k w) -> p h k w", h=2, k=2)
                    nc.scalar.activation(eout, cin, AF.Exp)
                # lse path first (and scheduler-prioritized) so the ACT Ln
                # chain unblocks as early as possible
                with tc.high_priority(offset=6):
                    nc.vector.tensor_tensor(
                        s2[:], ea[:, : 2 * W], ea[:, 2 * W :], AluOpType.add
                    )
                    nc.vector.tensor_tensor(
                        s[:], s2[:, :W], s2[:, W:], AluOpType.add
                    )
                    lw = W
                    bufs = (s, sp)
                    for _d in range(prod_depth):
                        lw //= 2
                        src, dst = bufs[_d % 2][:], bufs[(_d + 1) % 2][:]
                        nc.vector.tensor_tensor(
                            dst[:, :lw], src[:, :lw], src[:, lw : 2 * lw],
                            AluOpType.mult,
                        )
                    nc.scalar.activation(
                        s2[:, :lw], bufs[prod_depth % 2][:, :lw], AF.Ln,
                        accum_out=acc[:, t : t + 1],
                    )
                # selection
                nc.vector.tensor_tensor(
                    pm[:], lga[:, W:], lga[:, :W], AluOpType.is_gt
                )
                c16 = ca.bitcast(I16)
                nc.vector.copy_predicated(c16[:, :W], pm[:], c16[:, W:])
                apairs = ca[:, : 2 * W].rearrange("p (w k) -> p k w", k=2)
                sel_ap = apairs[:, 1]
                nc.vector.copy_predicated(sel_ap, tga, apairs[:, 0])
                for c in range(n_chunks):
                    lo, hi = c * 512, min((c + 1) * 512, W)
                    nc.tensor.matmul(
                        selp[:, : hi - lo],
                        ones[:],
                        sel_ap[:, lo:hi],
                        start=(ic == 0),
                        stop=(ic == n_chunks * T_ - 1),
                    )
                    ic += 1
            seljunk = acc_pool.tile([1, 512], F32)
            nc.scalar.activation(
                seljunk[:], selp[:], AF.Copy,
                accum_out=acc[0:1, T_ : T_ + 1],
            )
            nc.sync.dma_start(out=out_acc, in_=acc[:])
    return nc


def build_nc_v7(n_tiles=T, width=None, tmp_bufs=2, prod_depth=1, split_t0=True,
                io_bufs=None):
    if width is None:
        width = NP_CORE // (n_tiles * P)
    total = n_tiles * width
    nc = bacc.Bacc("TRN2", target_bir_lowering=False, debug=False)
    conf = nc.dram_tensor("conf", [P, 4 * total], FP8, kind="ExternalInput").ap()
    lg = nc.dram_tensor("lg", [P, 2 * total], BF16, kind="ExternalInput").ap()
    tgt = nc.dram_tensor("tgt", [P, total], U8, kind="ExternalInput").ap()
    out_acc = nc.dram_tensor(
        "acc", [P, n_tiles + 1], F32, kind="ExternalOutput"
    ).ap()
    emit_loss_kernel_v7(
        nc, conf, lg, tgt, out_acc, n_tiles, width, tmp_bufs, prod_depth, split_t0
    )
    nc.finalize()
    _pin_act_table_set(nc, 6)
    return nc


def shard_inputs_v7(pred_logits, pred_confusion, target_classes, n_tiles=T):
    width = NP_CORE // (n_tiles * P)
    widths = tuple([width] * n_tiles)
    return shard_inputs_v5(
        pred_logits, pred_confusion, target_classes, widths=widths
    )


def reduce_v7(results, n_tiles=T):
    total = 0.0
    for r in results:
        a = np.asarray(r["acc"], dtype=np.float64)
        total += a[:, :n_tiles].sum() - a[0, n_tiles]
    return np.float32(total / (B * N))


def kernel_v7(pred_logits, pred_confusion, target_classes):
    if "nc7" not in _CACHED:
        _CACHED["nc7"] = build_nc_v7()
    in_maps = shard_inputs_v7(pred_logits, pred_confusion, target_classes)
    results = run_bass_kernel_spmd(_CACHED["nc7"], in_maps, list(range(M))).results
    return reduce_v7(results)


_CACHED = {}


def _get_nc():
    if "nc" not in _CACHED:
        _CACHED["nc"] = build_nc()
    return _CACHED["nc"]


def kernel(pred_logits, pred_confusion, target_classes):
    nc = _get_nc()
    in_maps = shard_inputs(pred_logits, pred_confusion, target_classes)
    results = run_bass_kernel_spmd(nc, in_maps, list(range(M))).results
    total = 0.0
    for r in results:
        a = np.asarray(r["acc"], dtype=np.float64)
        total += a[:, :T].sum() - a[:, T:].sum()
    return np.float32(total / (B * N))


def reduce_v2(results):
    total = 0.0
    for r in results:
        a = np.asarray(r["acc"], dtype=np.float64)
        total += a[:, :T].sum() - a[0, T] - a[:, T + 1 :].sum()
    return np.float32(total / (B * N))


def kernel_v2(pred_logits, pred_confusion, target_classes):
    if "nc2" not in _CACHED:
        _CACHED["nc2"] = build_nc_v2()
    in_maps = shard_inputs_v2(pred_logits, pred_confusion, target_classes)
    results = run_bass_kernel_spmd(_CACHED["nc2"], in_maps, list(range(M))).results
    return reduce_v2(results)

